# revision 1
# baseline (speedup 1.0000x reference)
"""Trainium2 Bass kernel for a decoder layer (LN->attn->res, LN->attn->FFN->res).

Sharding (8 cores, SPMD single program):
 - Row-parallel phases (LN / out-proj / FFN / residuals): global rows = B*S =
   4096 split 512/core: core c owns batch c//4, seq rows [512*(c%4), +512).
 - Attention core phase: head-parallel: core c computes 2 global heads
   {2c, 2c+1} for BOTH batches over the full sequence (rank-uniform program;
   the rank-dependence lives in host-sliced per-core weight columns).
 - Collectives: two 8-core AllGathers per attention (LN output, attention
   output); attention-output readback uses partition-id dynamic DMA slices.

Layout: activations kept feature-on-partition ("transposed", [D, rows]) all
the way through; host pre-transposes x and post-transposes the output.
Matmul operands bf16, accumulation fp32, residual stream fp32.

Masking: the reference masks k <= q (strictly-upper-triangular attention) and
softmax rows are bounded, so softmax runs without max-subtraction; diagonal
tiles get an additive -60 mask pre-exp; fully-masked tiles are skipped. The
all-masked last row (uniform attention over all 2048 keys) is reproduced
exactly via a ones-column appended to V (denominators for free in PSUM row
64) plus ones-weight matmuls for the skipped tiles of the last query column.
"""

import numpy as np
import ml_dtypes

import concourse.bass as bass
import concourse.bacc as bacc
import concourse.tile as tile
import concourse.mybir as mybir
from concourse import bass_utils
from concourse.masks import make_identity

B, S, D, H, F = 2, 2048, 1024, 16, 4096
HD = D // H            # 64
NCORES = 8
R = S // 4             # 512 rows per core
KT16 = S // 128        # 16 seq tiles
DT8 = D // 128         # 8 feature tiles of D
FT32 = F // 128        # 32 feature tiles of F
EPS = 1e-5
NEG = -60.0

f32 = mybir.dt.float32
bf16 = mybir.dt.bfloat16
GROUP8 = [list(range(8))]
AF = mybir.ActivationFunctionType
ALU = mybir.AluOpType


def _emit(nc, tc, ctxpools):
    pid = nc.partition_id()
    bc = pid // 4          # my batch
    qc = pid % 4           # my seq chunk within batch


    # ---------------- I/O tensors ----------------
    xT = nc.dram_tensor("xT", [D, R], f32, kind="ExternalInput")
    wq_my = nc.dram_tensor("wq_my", [D, 2 * HD], bf16, kind="ExternalInput")
    wk_my = nc.dram_tensor("wk_my", [D, 2 * HD], bf16, kind="ExternalInput")
    wv_my = nc.dram_tensor("wv_my", [D, 2 * HD], bf16, kind="ExternalInput")
    wo = nc.dram_tensor("wo", [D, D], bf16, kind="ExternalInput")
    w1 = nc.dram_tensor("w1", [D, F], bf16, kind="ExternalInput")
    w2 = nc.dram_tensor("w2", [F, D], bf16, kind="ExternalInput")
    bo_c = nc.dram_tensor("bo_c", [D, 1], f32, kind="ExternalInput")
    b1_c = nc.dram_tensor("b1_c", [F, 1], f32, kind="ExternalInput")
    b2_c = nc.dram_tensor("b2_c", [D, 1], f32, kind="ExternalInput")
    g1_c = nc.dram_tensor("g1_c", [D, 1], f32, kind="ExternalInput")
    bln1_c = nc.dram_tensor("bln1_c", [D, 1], f32, kind="ExternalInput")
    g2_c = nc.dram_tensor("g2_c", [D, 1], f32, kind="ExternalInput")
    bln2_c = nc.dram_tensor("bln2_c", [D, 1], f32, kind="ExternalInput")
    tri = nc.dram_tensor("tri", [128, 128], bf16, kind="ExternalInput")
    outT = nc.dram_tensor("outT", [D, R], f32, kind="ExternalOutput")

    const = ctxpools["const"]
    persist = ctxpools["persist"]
    dram = ctxpools["dram"]

    # ---------------- constants / weights resident in SBUF ----------------
    wo_sb = const.tile([128, DT8, D], bf16)
    nc.sync.dma_start(out=wo_sb, in_=wo[:, :].rearrange("(k p) n -> p k n", p=128))
    wq_sb = const.tile([128, DT8, 2 * HD], bf16)
    nc.sync.dma_start(out=wq_sb, in_=wq_my[:, :].rearrange("(k p) n -> p k n", p=128))
    wk_sb = const.tile([128, DT8, 2 * HD], bf16)
    nc.sync.dma_start(out=wk_sb, in_=wk_my[:, :].rearrange("(k p) n -> p k n", p=128))
    wv_sb = const.tile([128, DT8, 2 * HD], bf16)
    nc.sync.dma_start(out=wv_sb, in_=wv_my[:, :].rearrange("(k p) n -> p k n", p=128))
    bo_sb = const.tile([128, DT8], f32)
    nc.sync.dma_start(out=bo_sb, in_=bo_c[:, 0:1].rearrange("(k p) o -> p (k o)", p=128))
    b1_sb = const.tile([128, FT32], f32)
    nc.sync.dma_start(out=b1_sb, in_=b1_c[:, 0:1].rearrange("(k p) o -> p (k o)", p=128))
    b2_sb = const.tile([128, DT8], f32)
    nc.sync.dma_start(out=b2_sb, in_=b2_c[:, 0:1].rearrange("(k p) o -> p (k o)", p=128))
    g1_sb = const.tile([128, DT8], f32)
    nc.sync.dma_start(out=g1_sb, in_=g1_c[:, 0:1].rearrange("(k p) o -> p (k o)", p=128))
    bln1_sb = const.tile([128, DT8], f32)
    nc.sync.dma_start(out=bln1_sb, in_=bln1_c[:, 0:1].rearrange("(k p) o -> p (k o)", p=128))
    g2_sb = const.tile([128, DT8], f32)
    nc.sync.dma_start(out=g2_sb, in_=g2_c[:, 0:1].rearrange("(k p) o -> p (k o)", p=128))
    bln2_sb = const.tile([128, DT8], f32)
    nc.sync.dma_start(out=bln2_sb, in_=bln2_c[:, 0:1].rearrange("(k p) o -> p (k o)", p=128))
    tri_sb = const.tile([128, 128], bf16)
    nc.sync.dma_start(out=tri_sb, in_=tri[:, :])
    ident_bf = const.tile([128, 128], bf16)
    make_identity(nc, ident_bf)
    ones_row = const.tile([1, HD], f32)
    nc.vector.memset(ones_row, 1.0)
    ones_bf = const.tile([128, 1], bf16)
    nc.vector.memset(ones_bf, 1.0)
    eps_sb = const.tile([1, 1], f32)
    nc.vector.memset(eps_sb, EPS)

    # ---------------- persistent activations ----------------
    resT = persist.tile([128, DT8, R], f32)    # residual stream (fp32)
    h2T = persist.tile([128, DT8, R], bf16)    # attn2 out -> FFN in
    OT = persist.tile([128, DT8, R], bf16)     # gathered attention out

    nc.sync.dma_start(out=resT, in_=xT[:, :].rearrange("(k p) q -> p k q", p=128))

    # =================================================================
    def layernorm(g_sb, bln_sb, hT):
        with (
            tc.tile_pool(name="ln_sb", bufs=2) as lnp,
            tc.tile_pool(name="ln_ps", bufs=2, space="PSUM") as lnps,
        ):
            ps_sum = lnps.tile([1, R], f32, tag="s")
            ps_sq = lnps.tile([1, R], f32, tag="q")
            for kt in range(DT8):
                rb = lnp.tile([128, R], bf16, tag="rb", bufs=3)
                nc.vector.tensor_copy(out=rb, in_=resT[:, kt, :])
                sq = lnp.tile([128, R], bf16, tag="sq", bufs=3)
                nc.vector.tensor_mul(out=sq, in0=rb, in1=rb)
                nc.tensor.matmul(ps_sum, ones_bf, rb,
                                 start=(kt == 0), stop=(kt == DT8 - 1))
                nc.tensor.matmul(ps_sq, ones_bf, sq,
                                 start=(kt == 0), stop=(kt == DT8 - 1))
            mu = lnp.tile([1, R], f32)
            nc.vector.tensor_scalar_mul(out=mu, in0=ps_sum, scalar1=1.0 / D)
            msq = lnp.tile([1, R], f32)
            nc.vector.tensor_scalar_mul(out=msq, in0=ps_sq, scalar1=1.0 / D)
            mu2 = lnp.tile([1, R], f32)
            nc.vector.tensor_mul(out=mu2, in0=mu, in1=mu)
            var = lnp.tile([1, R], f32)
            nc.vector.tensor_tensor(out=var, in0=msq, in1=mu2, op=ALU.subtract)
            sd = lnp.tile([1, R], f32)
            nc.scalar.activation(out=sd, in_=var, func=AF.Sqrt, bias=eps_sb)
            rstd = lnp.tile([1, R], f32)
            nc.vector.reciprocal(out=rstd, in_=sd)
            scr = dram.tile([2, R], f32, tag="lnscr", bufs=2)
            nc.sync.dma_start(out=scr[0:1, :], in_=mu)
            nc.sync.dma_start(out=scr[1:2, :], in_=rstd)
            mr = lnp.tile([128, 2, R], f32)
            nc.sync.dma_start(out=mr, in_=scr[None].broadcast_to([128, 2, R]))
            for kt in range(DT8):
                t1 = lnp.tile([128, R], f32, tag="t1", bufs=3)
                nc.vector.tensor_tensor(out=t1, in0=resT[:, kt, :],
                                        in1=mr[:, 0, :], op=ALU.subtract)
                t2 = lnp.tile([128, R], f32, tag="t2", bufs=3)
                nc.vector.tensor_mul(out=t2, in0=t1, in1=mr[:, 1, :])
                nc.vector.tensor_scalar(out=hT[:, kt, :], in0=t2,
                                        scalar1=g_sb[:, kt:kt + 1],
                                        scalar2=bln_sb[:, kt:kt + 1],
                                        op0=ALU.mult, op1=ALU.add)

    # =================================================================
    def attention(hT, first):
        # AllGather the LN output in two halves (feature tiles 0-3 / 4-7) so
        # the first collective overlaps the LN tail and QKV accumulation
        # starts as soon as the first half lands.
        ag1_in = dram.tile([D, R], bf16, tag="ag1i", bufs=2)
        ag1_out = dram.tile([NCORES * D, R], bf16, addr_space="Shared",
                            tag="ag1o", bufs=2)
        nc.sync.dma_start(
            out=ag1_in.rearrange("(k p) q -> p k q", p=128), in_=hT)
        nc.gpsimd.collective_compute(
            "AllGather", ALU.bypass, replica_groups=GROUP8,
            ins=[ag1_in.opt()], outs=[ag1_out.opt()])
        ag1v = ag1_out.rearrange("(r k p) q -> r k p q", r=NCORES, k=DT8, p=128)

        # attention-output gather: one collective per batch (batch 0's hides
        # behind batch 1's attention), concatenated Shared output so the
        # readback is a single rank-uniform dynamic slice.
        ag2_in = dram.tile([B * 128, S], bf16, tag="ag2i", bufs=2)
        ag2_out = dram.tile([NCORES * B * 128, S], bf16, addr_space="Shared",
                            tag="ag2o", bufs=2)

        with (
            tc.tile_pool(name="at_sb", bufs=2) as ap,
            tc.tile_pool(name="at_big", bufs=2) as bigp,
        ):
            for b in range(B):
                # ---- gather LN output for batch b: [128, kt, 2048] ----
                hfull = bigp.tile([128, DT8, S], bf16, tag="hfull")
                for kt in range(DT8):
                    nc.sync.dma_start(
                        out=hfull[:, kt, :].rearrange("p (r q) -> p r q", r=4),
                        in_=ag1v[4 * b:4 * b + 4, kt].rearrange(
                            "r p q -> p r q"))

                # ---- QKV for my 2 heads, full sequence ----
                qT = ap.tile([128, 4, R], bf16, tag="qT")
                kT = ap.tile([128, 4, R], bf16, tag="kT")
                vT = ap.tile([128, 4, R], bf16, tag="vT")
                with tc.tile_pool(name="qkv_ps", bufs=4, space="PSUM") as qps:
                    for dst, wsb, scale in ((qT, wq_sb, 0.125),
                                            (kT, wk_sb, None),
                                            (vT, wv_sb, None)):
                        for c in range(4):
                            ps = qps.tile([128, R], f32, tag="ps")
                            for kt in range(DT8):
                                nc.tensor.matmul(
                                    ps, wsb[:, kt, :],
                                    hfull[:, kt, c * R:(c + 1) * R],
                                    start=(kt == 0), stop=(kt == DT8 - 1))
                            if scale is None:
                                nc.vector.tensor_copy(out=dst[:, c, :], in_=ps)
                            else:
                                nc.vector.tensor_scalar_mul(
                                    out=dst[:, c, :], in0=ps, scalar1=scale)

                # ---- V transpose -> v_aug [128(seq), kt, h, 65] ----
                vaug = ap.tile([128, KT16, 2, HD + 1], bf16, tag="vaug")
                nc.vector.memset(vaug[:, :, :, HD:HD + 1], 1.0)
                for kt in range(KT16):
                    vtmp = ap.tile([128, 128], bf16, tag="vtmp", bufs=4)
                    nc.sync.dma_start(
                        out=vtmp,
                        in_=vT[:, kt // 4, (kt % 4) * 128:(kt % 4) * 128 + 128],
                        transpose=True)
                    nc.vector.tensor_copy(out=vaug[:, kt, 0, 0:HD],
                                          in_=vtmp[:, 0:HD])
                    nc.vector.tensor_copy(out=vaug[:, kt, 1, 0:HD],
                                          in_=vtmp[:, HD:128])

                # ---- attention core ----
                oT = ap.tile([128, 4, R], bf16, tag="oT")
                with (
                    tc.tile_pool(name="sc_ps", bufs=4, space="PSUM") as scps,
                    tc.tile_pool(name="av_ps", bufs=4, space="PSUM") as avps,
                ):
                    for c in range(4):
                        po = [avps.tile([128, R], f32, tag="po", name=f"po{hh}")
                              for hh in range(2)]
                        for kt in range(KT16 - 1, 4 * c - 1, -1):
                            npfx = min(kt - 4 * c + 1, 4) * 128
                            pss = [scps.tile([128, R], f32, tag="pss",
                                             name=f"pss{hh}", bufs=4)
                                   for hh in range(2)]
                            wts = []
                            for hh in range(2):
                                lo, hi = hh * HD, hh * HD + HD
                                diag = kt <= 4 * c + 3
                                nc.tensor.matmul(
                                    pss[hh][:, 0:npfx],
                                    kT[lo:hi, kt // 4,
                                       (kt % 4) * 128:(kt % 4) * 128 + 128],
                                    qT[lo:hi, c, 0:npfx],
                                    start=True, stop=not diag)
                                if diag:
                                    nc.tensor.matmul(
                                        pss[hh][:, npfx - 128:npfx],
                                        ident_bf, tri_sb,
                                        start=False, stop=True,
                                        skip_group_check=True)
                                wt = ap.tile([128, R], bf16, tag="wt",
                                             name=f"wt{hh}", bufs=4)
                                nc.scalar.activation(out=wt[:, 0:npfx],
                                                     in_=pss[hh][:, 0:npfx],
                                                     func=AF.Exp)
                                if c == 3 and kt == KT16 - 1:
                                    nc.vector.memset(wt[:, R - 1:R], 1.0)
                                wts.append(wt)
                            for hh in range(2):
                                nc.tensor.matmul(
                                    po[hh][0:HD + 1, 0:npfx],
                                    vaug[:, kt, hh, :],
                                    wts[hh][:, 0:npfx],
                                    start=(kt == KT16 - 1),
                                    stop=(c < 3 and kt == 4 * c))
                        if c == 3:
                            # uniform last row: accumulate skipped tiles with
                            # weight 1 into the final column
                            for kt in range(KT16 - 2, -1, -1):
                                for hh in range(2):
                                    nc.tensor.matmul(
                                        po[hh][0:HD + 1, R - 1:R],
                                        vaug[:, kt, hh, :], ones_bf,
                                        start=False, stop=(kt == 0),
                                        skip_group_check=True)
                        # normalize by PSUM row HD (the softmax denominators)
                        scr2 = dram.tile([2, R], f32, tag="rscr", bufs=4)
                        for hh in range(2):
                            rec = ap.tile([1, R], f32, tag="rec",
                                          name=f"rec{hh}", bufs=4)
                            nc.vector.reciprocal(out=rec, in_=po[hh][HD:HD + 1, :])
                            nc.sync.dma_start(out=scr2[hh:hh + 1, :], in_=rec)
                        for hh in range(2):
                            rbt = ap.tile([HD, R], f32, tag="rb",
                                          name=f"rb{hh}", bufs=4)
                            nc.sync.dma_start(
                                out=rbt,
                                in_=scr2[hh, :][None].broadcast_to([HD, R]))
                            nc.vector.tensor_mul(
                                out=oT[hh * HD:hh * HD + HD, c, :],
                                in0=po[hh][0:HD, :], in1=rbt)
                nc.sync.dma_start(
                    out=ag2_in[b * 128:(b + 1) * 128, :].rearrange(
                        "p (c q) -> p c q", c=4),
                    in_=oT)
        nc.gpsimd.collective_compute(
            "AllGather", ALU.bypass, replica_groups=GROUP8,
            ins=[ag2_in.opt()], outs=[ag2_out.opt()])
        # ---- readback my rows: per sender s, its 2 heads, my (batch, chunk)
        for s in range(NCORES):
            nc.sync.dma_start(
                out=OT[:, s, :],
                in_=ag2_out[bass.ds(s * B * 128 + bc * 128, 128),
                            bass.ds(qc * R, R)])

        # ---- out-projection (+ bias, + residual or -> h2T) ----
        with (
            tc.tile_pool(name="op_sb", bufs=3) as opp,
            tc.tile_pool(name="op_ps", bufs=4, space="PSUM") as opps,
        ):
            for nt in range(DT8):
                ps = opps.tile([128, R], f32, tag="ps")
                for s in range(DT8):
                    nc.tensor.matmul(ps, wo_sb[:, s, nt * 128:nt * 128 + 128],
                                     OT[:, s, :],
                                     start=(s == 0), stop=(s == DT8 - 1))
                if first:
                    t = opp.tile([128, R], f32, tag="t")
                    nc.vector.tensor_scalar(out=t, in0=ps,
                                            scalar1=bo_sb[:, nt:nt + 1],
                                            scalar2=None, op0=ALU.add)
                    nc.vector.tensor_add(out=resT[:, nt, :],
                                         in0=resT[:, nt, :], in1=t)
                else:
                    nc.vector.tensor_scalar(out=h2T[:, nt, :], in0=ps,
                                            scalar1=bo_sb[:, nt:nt + 1],
                                            scalar2=None, op0=ALU.add)

    # =================================================================
    # layer body
    hT1 = persist.tile([128, DT8, R], bf16)
    layernorm(g1_sb, bln1_sb, hT1)
    attention(hT1, first=True)
    layernorm(g2_sb, bln2_sb, hT1)
    attention(hT1, first=False)

    # ---- FFN ----
    with (
        tc.tile_pool(name="ffn_sb", bufs=3) as fp,
        tc.tile_pool(name="ffn_big", bufs=1) as fbig,
        tc.tile_pool(name="ffn_ps", bufs=4, space="PSUM") as fps,
    ):
        gT = fbig.tile([128, FT32, R], bf16)
        w1v = w1[:, :].rearrange("(k p) (nt n) -> nt k p n", p=128, n=128)
        for nt in range(FT32):
            w1t = fp.tile([128, DT8, 128], bf16, tag="w1t", bufs=4)
            nc.sync.dma_start(out=w1t, in_=w1v[nt].rearrange("k p n -> p k n"))
            ps = fps.tile([128, R], f32, tag="ps1")
            for kt in range(DT8):
                nc.tensor.matmul(ps, w1t[:, kt, :], h2T[:, kt, :],
                                 start=(kt == 0), stop=(kt == DT8 - 1))
            nc.scalar.activation(out=gT[:, nt, :], in_=ps, func=AF.Gelu,
                                 bias=b1_sb[:, nt:nt + 1])
        w2v = w2[:, :].rearrange("(k p) (nt n) -> nt k p n", p=128, n=128)
        for nt in range(DT8):
            w2t = fp.tile([128, FT32, 128], bf16, tag="w2t", bufs=3)
            nc.sync.dma_start(out=w2t, in_=w2v[nt].rearrange("k p n -> p k n"))
            ps2 = fps.tile([128, R], f32, tag="ps2")
            for kt in range(FT32):
                nc.tensor.matmul(ps2, w2t[:, kt, :], gT[:, kt, :],
                                 start=(kt == 0), stop=(kt == FT32 - 1))
            t = fp.tile([128, R], f32, tag="t")
            nc.vector.tensor_scalar(out=t, in0=ps2,
                                    scalar1=b2_sb[:, nt:nt + 1],
                                    scalar2=None, op0=ALU.add)
            ot = fp.tile([128, R], f32, tag="ot")
            nc.vector.tensor_add(out=ot, in0=t, in1=resT[:, nt, :])
            nc.sync.dma_start(out=outT[nt * 128:(nt + 1) * 128, :], in_=ot)


def build():
    nc = bacc.Bacc("TRN2", target_bir_lowering=False, debug=False,
                   num_devices=NCORES)
    with tile.TileContext(nc) as tc:
        with (
            tc.tile_pool(name="const", bufs=1) as const,
            tc.tile_pool(name="persist", bufs=1) as persist,
            tc.tile_pool(name="dram", bufs=1, space="DRAM") as dram,
        ):
            _emit(nc, tc, {"const": const, "persist": persist, "dram": dram})
    nc.compile()
    return nc


_CACHED = {}


def _get_nc():
    if "nc" not in _CACHED:
        _CACHED["nc"] = build()
    return _CACHED["nc"]


def _prep_in_maps(inputs):
    gf = lambda k: np.asarray(inputs[k], np.float32)
    x = gf("x")
    wq, wk, wv, wo_ = gf("wq"), gf("wk"), gf("wv"), gf("wo")
    w1_, w2_ = gf("w1"), gf("w2")
    tobf = lambda a: np.ascontiguousarray(a).astype(ml_dtypes.bfloat16)
    wo_b, w1_b, w2_b = tobf(wo_), tobf(w1_), tobf(w2_)
    col = lambda a: np.ascontiguousarray(gf(a).reshape(-1, 1))
    tri_np = np.where(np.arange(128)[:, None] <= np.arange(128)[None, :],
                      np.float32(NEG), np.float32(0.0)).astype(ml_dtypes.bfloat16)
    shared = dict(wo=wo_b, w1=w1_b, w2=w2_b,
                  bo_c=col("bo"), b1_c=col("b1"), b2_c=col("b2"),
                  g1_c=col("ln1_g"), bln1_c=col("ln1_b"),
                  g2_c=col("ln2_g"), bln2_c=col("ln2_b"), tri=tri_np)
    in_maps = []
    for c in range(NCORES):
        b, q = c // 4, c % 4
        m = dict(shared)
        m["xT"] = np.ascontiguousarray(x[b, q * R:(q + 1) * R, :].T)
        m["wq_my"] = tobf(wq[:, 128 * c:128 * (c + 1)])
        m["wk_my"] = tobf(wk[:, 128 * c:128 * (c + 1)])
        m["wv_my"] = tobf(wv[:, 128 * c:128 * (c + 1)])
        in_maps.append(m)
    return in_maps


def run(inputs, **kw):
    nc = _get_nc()
    in_maps = _prep_in_maps(inputs)
    res = bass_utils.run_bass_kernel_spmd(nc, in_maps,
                                          core_ids=list(range(NCORES)), **kw)
    out = np.empty((B, S, D), np.float32)
    for c in range(NCORES):
        b, q = c // 4, c % 4
        out[b, q * R:(q + 1) * R, :] = res.results[c]["outT"].T
    return out, res


def kernel(**inputs):
    out, _ = run(inputs)
    return out



# revision 5
# speedup vs baseline: 1.1178x; 1.1178x over previous
"""Trainium2 Bass kernel for a decoder layer (LN->attn->res, LN->attn->FFN->res).

Sharding (8 cores, SPMD single program, fully rank-uniform IR):
 - Row-parallel phases (LN / QKV / out-proj / FFN / residuals): global rows
   B*S = 4096 split 512/core: core c owns batch c//4, seq rows [512*(c%4), +512).
 - QKV computed data-parallel (each core: all 1024 features of its own 512
   rows), then redistributed head-parallel with three AllToAlls (k, q, v; each
   1 MB/core) that pipeline behind the projection matmuls. Attention core:
   head-parallel (2 heads/core over the full sequence, per batch).
 - Attention outputs return to row-parallel with one more AllToAll (1 MB/core)
   feeding a local out-projection. No AllGathers, no AllReduce: 4 collectives
   per attention with ~8x less wire than gather-based tensor parallelism.

Layout: activations feature-on-partition ([D, rows]); host pre-transposes x
and post-transposes the output. Matmuls bf16, accumulation fp32, residual fp32.

Masking: reference masks k <= q (strictly-upper attention). Softmax runs
without max-subtraction (scores bounded); diagonal 128x128 blocks are masked
multiplicatively AFTER exp with a 0/1 upper-strict mask on the vector engine
(exact zeros, no PE matmul); fully-masked tiles are skipped via per-tile
column prefixes. Softmax denominators ride in PSUM row 64 via a ones-column
appended to V. The all-masked last row (uniform attention over all 2048 keys)
is patched post-normalize with a DVE reduce of V over the full sequence.

LayerNorm stats are computed with a [128,128] ones stationary so every
per-row scalar op runs on all 128 lanes (no single-lane reciprocal/sqrt,
no DMA broadcast round-trip).
"""

import numpy as np
import ml_dtypes

import concourse.bass as bass
import concourse.bacc as bacc
import concourse.tile as tile
import concourse.mybir as mybir
from concourse import bass_utils

B, S, D, H, F = 2, 2048, 1024, 16, 4096
HD = D // H            # 64
NCORES = 8
R = S // 4             # 512 rows per core
KT16 = S // 128        # 16 seq tiles
DT8 = D // 128         # 8 feature tiles of D
FT32 = F // 128        # 32 feature tiles of F
EPS = 1e-5

f32 = mybir.dt.float32
bf16 = mybir.dt.bfloat16
GROUP8 = [list(range(8))]
AF = mybir.ActivationFunctionType
ALU = mybir.AluOpType


def _emit(nc, tc, ctxpools):
    # ---------------- I/O tensors ----------------
    xT = nc.dram_tensor("xT", [D, R], f32, kind="ExternalInput")
    wq = nc.dram_tensor("wq", [D, D], bf16, kind="ExternalInput")
    wk = nc.dram_tensor("wk", [D, D], bf16, kind="ExternalInput")
    wv = nc.dram_tensor("wv", [D, D], bf16, kind="ExternalInput")
    wo = nc.dram_tensor("wo", [D, D], bf16, kind="ExternalInput")
    w1 = nc.dram_tensor("w1", [D, F], bf16, kind="ExternalInput")
    w2 = nc.dram_tensor("w2", [F, D], bf16, kind="ExternalInput")
    bo_c = nc.dram_tensor("bo_c", [D, 1], f32, kind="ExternalInput")
    b1_c = nc.dram_tensor("b1_c", [F, 1], f32, kind="ExternalInput")
    b2_c = nc.dram_tensor("b2_c", [D, 1], f32, kind="ExternalInput")
    g1_c = nc.dram_tensor("g1_c", [D, 1], f32, kind="ExternalInput")
    bln1_c = nc.dram_tensor("bln1_c", [D, 1], f32, kind="ExternalInput")
    g2_c = nc.dram_tensor("g2_c", [D, 1], f32, kind="ExternalInput")
    bln2_c = nc.dram_tensor("bln2_c", [D, 1], f32, kind="ExternalInput")
    tri2 = nc.dram_tensor("tri2", [128, 2, 128], bf16, kind="ExternalInput")
    outT = nc.dram_tensor("outT", [D, R], f32, kind="ExternalOutput")

    const = ctxpools["const"]
    persist = ctxpools["persist"]
    dram = ctxpools["dram"]

    # ---------------- persistent activations ----------------
    resT = persist.tile([128, DT8, R], f32)    # residual stream (fp32)
    hT1 = persist.tile([128, DT8, R], bf16)    # LN output / attn input
    h2T = persist.tile([128, DT8, R], bf16)    # attn2 out -> FFN in
    OT = persist.tile([128, DT8, R], bf16)     # gathered attention out

    # x first: LN1 is the first consumer
    nc.sync.dma_start(out=resT, in_=xT[:, :].rearrange("(k p) q -> p k q", p=128))

    # ---------------- constants / weights resident in SBUF ----------------
    g1_sb = const.tile([128, DT8], f32)
    nc.sync.dma_start(out=g1_sb, in_=g1_c[:, 0:1].rearrange("(k p) o -> p (k o)", p=128))
    bln1_sb = const.tile([128, DT8], f32)
    nc.sync.dma_start(out=bln1_sb, in_=bln1_c[:, 0:1].rearrange("(k p) o -> p (k o)", p=128))
    g2_sb = const.tile([128, DT8], f32)
    nc.sync.dma_start(out=g2_sb, in_=g2_c[:, 0:1].rearrange("(k p) o -> p (k o)", p=128))
    bln2_sb = const.tile([128, DT8], f32)
    nc.sync.dma_start(out=bln2_sb, in_=bln2_c[:, 0:1].rearrange("(k p) o -> p (k o)", p=128))
    bo_sb = const.tile([128, DT8], f32)
    nc.sync.dma_start(out=bo_sb, in_=bo_c[:, 0:1].rearrange("(k p) o -> p (k o)", p=128))
    b1_sb = const.tile([128, FT32], f32)
    nc.sync.dma_start(out=b1_sb, in_=b1_c[:, 0:1].rearrange("(k p) o -> p (k o)", p=128))
    b2_sb = const.tile([128, DT8], f32)
    nc.sync.dma_start(out=b2_sb, in_=b2_c[:, 0:1].rearrange("(k p) o -> p (k o)", p=128))
    tri2_sb = const.tile([128, 2, 128], bf16)
    nc.sync.dma_start(out=tri2_sb, in_=tri2[:, :, :])
    ones128 = const.tile([128, 128], bf16)
    nc.vector.memset(ones128, 1.0)
    eps_sb = const.tile([128, 1], f32)
    nc.vector.memset(eps_sb, EPS)
    # QKV weights in projection order (k, q, v), then wo
    wk_sb = const.tile([128, DT8, D], bf16)
    nc.sync.dma_start(out=wk_sb, in_=wk[:, :].rearrange("(k p) n -> p k n", p=128))
    wq_sb = const.tile([128, DT8, D], bf16)
    nc.sync.dma_start(out=wq_sb, in_=wq[:, :].rearrange("(k p) n -> p k n", p=128))
    wv_sb = const.tile([128, DT8, D], bf16)
    nc.sync.dma_start(out=wv_sb, in_=wv[:, :].rearrange("(k p) n -> p k n", p=128))
    wo_sb = const.tile([128, DT8, D], bf16)
    nc.sync.dma_start(out=wo_sb, in_=wo[:, :].rearrange("(k p) n -> p k n", p=128))

    # =================================================================
    def layernorm(g_sb, bln_sb, hT):
        with (
            tc.tile_pool(name="ln_sb", bufs=2) as lnp,
            tc.tile_pool(name="ln_ps", bufs=1, space="PSUM") as lnps,
        ):
            ps_sum = lnps.tile([128, R], f32, tag="s")
            ps_sq = lnps.tile([128, R], f32, tag="q")
            for kt in range(DT8):
                rb = lnp.tile([128, R], bf16, tag="rb", bufs=3)
                nc.vector.tensor_copy(out=rb, in_=resT[:, kt, :])
                sq = lnp.tile([128, R], bf16, tag="sq", bufs=3)
                nc.vector.tensor_mul(out=sq, in0=rb, in1=rb)
                nc.tensor.matmul(ps_sum, ones128, rb,
                                 start=(kt == 0), stop=(kt == DT8 - 1))
                nc.tensor.matmul(ps_sq, ones128, sq,
                                 start=(kt == 0), stop=(kt == DT8 - 1))
            # all per-row scalars live on all 128 partitions (full DVE width)
            mu = lnp.tile([128, R], f32)
            nc.vector.tensor_scalar_mul(out=mu, in0=ps_sum, scalar1=1.0 / D)
            msq = lnp.tile([128, R], f32)
            nc.vector.tensor_scalar_mul(out=msq, in0=ps_sq, scalar1=1.0 / D)
            mu2 = lnp.tile([128, R], f32)
            nc.vector.tensor_mul(out=mu2, in0=mu, in1=mu)
            var = lnp.tile([128, R], f32)
            nc.vector.tensor_tensor(out=var, in0=msq, in1=mu2, op=ALU.subtract)
            sd = lnp.tile([128, R], f32)
            nc.scalar.activation(out=sd, in_=var, func=AF.Sqrt, bias=eps_sb)
            rstd = lnp.tile([128, R], f32)
            nc.vector.reciprocal(out=rstd, in_=sd)
            for kt in range(DT8):
                t1 = lnp.tile([128, R], f32, tag="t1", bufs=3)
                nc.vector.tensor_tensor(out=t1, in0=resT[:, kt, :],
                                        in1=mu, op=ALU.subtract)
                t2 = lnp.tile([128, R], f32, tag="t2", bufs=3)
                nc.vector.tensor_mul(out=t2, in0=t1, in1=rstd)
                nc.vector.tensor_scalar(out=hT[:, kt, :], in0=t2,
                                        scalar1=g_sb[:, kt:kt + 1],
                                        scalar2=bln_sb[:, kt:kt + 1],
                                        op0=ALU.mult, op1=ALU.add)

    # =================================================================
    def attention(hT, first):
        # -- data-parallel QKV over my 512 rows, all features; per-projection
        #    AllToAll redistributes to head-parallel and pipelines behind the
        #    next projection's matmuls.
        a2a_out = []
        with (
            tc.tile_pool(name="qkv_sb", bufs=2) as qsb,
            tc.tile_pool(name="qkv_ps", bufs=4, space="PSUM") as qps,
        ):
            for p, (wsb, scale) in enumerate(((wk_sb, None),
                                              (wq_sb, 0.125),
                                              (wv_sb, None))):
                st = qsb.tile([128, DT8, R], bf16, tag="st", name=f"st{p}")
                for nt in range(DT8):
                    ps = qps.tile([128, R], f32, tag="ps")
                    for kt in range(DT8):
                        nc.tensor.matmul(ps, wsb[:, kt, nt * 128:nt * 128 + 128],
                                         hT[:, kt, :],
                                         start=(kt == 0), stop=(kt == DT8 - 1))
                    if scale is None:
                        nc.vector.tensor_copy(out=st[:, nt, :], in_=ps)
                    else:
                        nc.vector.tensor_scalar_mul(out=st[:, nt, :], in0=ps,
                                                    scalar1=scale)
                ain = dram.tile([D, R], bf16, tag="a2ai", name=f"a2ai{p}", bufs=2)
                aout = dram.tile([D, R], bf16, tag="a2ao",
                                 name=f"a2ao{p}", bufs=2)
                nc.sync.dma_start(
                    out=ain.rearrange("(n p) q -> p n q", p=128), in_=st)
                nc.gpsimd.collective_compute(
                    "AllToAll", ALU.bypass, replica_groups=GROUP8,
                    ins=[ain.opt()], outs=[aout.opt()])
                a2a_out.append(aout)
        k_out, q_out, v_out = a2a_out

        # -- attention core: my 2 heads, both batches over full sequence
        a2a_o_in = dram.tile([NCORES * 128, R], bf16, tag="a2aoi", bufs=2)
        a2a_o_out = dram.tile([NCORES * 128, R], bf16, tag="a2aoo", bufs=2)
        with tc.tile_pool(name="at_sb", bufs=2) as ap:
            for b in range(B):
                qT = ap.tile([128, 4, R], bf16, tag="qT")
                kT = ap.tile([128, 4, R], bf16, tag="kT")
                vT = ap.tile([128, 4, R], bf16, tag="vT")
                for src, dst in ((k_out, kT), (q_out, qT), (v_out, vT)):
                    nc.sync.dma_start(
                        out=dst,
                        in_=src[bass.ds(4 * b * 128, 512), :].rearrange(
                            "(c p) q -> p c q", c=4))

                # V transpose -> v_aug [128(seq), kt, h, 65]
                vaug = ap.tile([128, KT16, 2, HD + 1], bf16, tag="vaug")
                nc.vector.memset(vaug[:, :, :, HD:HD + 1], 1.0)
                for kt in range(KT16):
                    vtmp = ap.tile([128, 128], bf16, tag="vtmp", bufs=4)
                    nc.sync.dma_start(
                        out=vtmp,
                        in_=vT[:, kt // 4, (kt % 4) * 128:(kt % 4) * 128 + 128],
                        transpose=True)
                    nc.vector.tensor_copy(out=vaug[:, kt, 0, 0:HD],
                                          in_=vtmp[:, 0:HD])
                    nc.vector.tensor_copy(out=vaug[:, kt, 1, 0:HD],
                                          in_=vtmp[:, HD:128])
                # mean of V over the whole sequence (for the all-masked last row)
                sumv = ap.tile([128, 1], f32, tag="sumv")
                nc.vector.tensor_reduce(out=sumv, in_=vT,
                                        axis=mybir.AxisListType.XY, op=ALU.add)

                oT = ap.tile([128, 4, R], bf16, tag="oT")
                with (
                    tc.tile_pool(name="sc_ps", bufs=2, space="PSUM") as scps,
                    tc.tile_pool(name="av_ps", bufs=2, space="PSUM") as avps,
                ):
                    for c in range(4):
                        po = [avps.tile([128, R], f32, tag="po", name=f"po{hh}")
                              for hh in range(2)]
                        for kt in range(KT16 - 1, 4 * c - 1, -1):
                            npfx = min(kt - 4 * c + 1, 4) * 128
                            pss = scps.tile([128, 2, R], f32, tag="pss")
                            for hh in range(2):
                                lo = hh * HD
                                nc.tensor.matmul(
                                    pss[:, hh, 0:npfx],
                                    kT[lo:lo + HD, kt // 4,
                                       (kt % 4) * 128:(kt % 4) * 128 + 128],
                                    qT[lo:lo + HD, c, 0:npfx],
                                    start=True, stop=True)
                            wt = ap.tile([128, 2, R], bf16, tag="wt", bufs=3)
                            nc.scalar.activation(out=wt[:, :, 0:npfx],
                                                 in_=pss[:, :, 0:npfx],
                                                 func=AF.Exp)
                            if kt <= 4 * c + 3:
                                # diagonal block: exact multiplicative mask
                                nc.vector.tensor_mul(
                                    out=wt[:, :, npfx - 128:npfx],
                                    in0=wt[:, :, npfx - 128:npfx],
                                    in1=tri2_sb)
                            for hh in range(2):
                                nc.tensor.matmul(
                                    po[hh][0:HD + 1, 0:npfx],
                                    vaug[:, kt, hh, :],
                                    wt[:, hh, 0:npfx],
                                    start=(kt == KT16 - 1),
                                    stop=(kt == 4 * c))
                        # normalize by PSUM row HD (softmax denominators)
                        den_sb = ap.tile([1, 2, R], f32, tag="densb", bufs=4)
                        for hh in range(2):
                            nc.vector.tensor_copy(out=den_sb[:, hh, :],
                                                  in_=po[hh][HD:HD + 1, :])
                        scr2 = dram.tile([1, 2, R], f32, tag="rscr", bufs=4)
                        nc.sync.dma_start(out=scr2, in_=den_sb)
                        denb = ap.tile([HD, 2, R], f32, tag="denb", bufs=2)
                        nc.sync.dma_start(
                            out=denb,
                            in_=scr2[0][None].broadcast_to([HD, 2, R]))
                        recb = ap.tile([HD, 2, R], f32, tag="recb", bufs=2)
                        nc.vector.reciprocal(out=recb, in_=denb)
                        for hh in range(2):
                            nc.vector.tensor_mul(
                                out=oT[hh * HD:hh * HD + HD, c, :],
                                in0=po[hh][0:HD, :], in1=recb[:, hh, :])
                        if c == 3:
                            # all-masked last row: uniform attention = mean(V)
                            nc.vector.tensor_scalar_mul(
                                out=oT[:, 3, R - 1:R], in0=sumv,
                                scalar1=1.0 / S)
                nc.sync.dma_start(
                    out=a2a_o_in[bass.ds(4 * b * 128, 512), :].rearrange(
                        "(c p) q -> p c q", c=4),
                    in_=oT)
        nc.gpsimd.collective_compute(
            "AllToAll", ALU.bypass, replica_groups=GROUP8,
            ins=[a2a_o_in.opt()], outs=[a2a_o_out.opt()])
        nc.sync.dma_start(
            out=OT, in_=a2a_o_out[:, :].rearrange("(s p) q -> p s q", p=128))

        # ---- out-projection (+ bias, + residual or -> h2T) ----
        with (
            tc.tile_pool(name="op_sb", bufs=3) as opp,
            tc.tile_pool(name="op_ps", bufs=4, space="PSUM") as opps,
        ):
            for nt in range(DT8):
                ps = opps.tile([128, R], f32, tag="ps")
                for s in range(DT8):
                    nc.tensor.matmul(ps, wo_sb[:, s, nt * 128:nt * 128 + 128],
                                     OT[:, s, :],
                                     start=(s == 0), stop=(s == DT8 - 1))
                if first:
                    t = opp.tile([128, R], f32, tag="t")
                    nc.vector.tensor_scalar(out=t, in0=ps,
                                            scalar1=bo_sb[:, nt:nt + 1],
                                            scalar2=None, op0=ALU.add)
                    nc.vector.tensor_add(out=resT[:, nt, :],
                                         in0=resT[:, nt, :], in1=t)
                else:
                    nc.vector.tensor_scalar(out=h2T[:, nt, :], in0=ps,
                                            scalar1=bo_sb[:, nt:nt + 1],
                                            scalar2=None, op0=ALU.add)

    # =================================================================
    # layer body
    layernorm(g1_sb, bln1_sb, hT1)
    attention(hT1, first=True)
    layernorm(g2_sb, bln2_sb, hT1)
    attention(hT1, first=False)

    # ---- FFN ----
    with (
        tc.tile_pool(name="ffn_sb", bufs=3) as fp,
        tc.tile_pool(name="ffn_big", bufs=1) as fbig,
        tc.tile_pool(name="ffn_ps", bufs=4, space="PSUM") as fps,
    ):
        gT = fbig.tile([128, FT32, R], bf16)
        w1v = w1[:, :].rearrange("(k p) (nt n) -> nt k p n", p=128, n=128)
        for nt in range(FT32):
            w1t = fp.tile([128, DT8, 128], bf16, tag="w1t", bufs=4)
            nc.sync.dma_start(out=w1t, in_=w1v[nt].rearrange("k p n -> p k n"))
            ps = fps.tile([128, R], f32, tag="ps1")
            for kt in range(DT8):
                nc.tensor.matmul(ps, w1t[:, kt, :], h2T[:, kt, :],
                                 start=(kt == 0), stop=(kt == DT8 - 1))
            nc.scalar.activation(out=gT[:, nt, :], in_=ps, func=AF.Gelu,
                                 bias=b1_sb[:, nt:nt + 1])
        w2v = w2[:, :].rearrange("(k p) (nt n) -> nt k p n", p=128, n=128)
        for nt in range(DT8):
            w2t = fp.tile([128, FT32, 128], bf16, tag="w2t", bufs=3)
            nc.sync.dma_start(out=w2t, in_=w2v[nt].rearrange("k p n -> p k n"))
            ps2 = fps.tile([128, R], f32, tag="ps2")
            for kt in range(FT32):
                nc.tensor.matmul(ps2, w2t[:, kt, :], gT[:, kt, :],
                                 start=(kt == 0), stop=(kt == FT32 - 1))
            t = fp.tile([128, R], f32, tag="t")
            nc.vector.tensor_scalar(out=t, in0=ps2,
                                    scalar1=b2_sb[:, nt:nt + 1],
                                    scalar2=None, op0=ALU.add)
            ot = fp.tile([128, R], f32, tag="ot")
            nc.vector.tensor_add(out=ot, in0=t, in1=resT[:, nt, :])
            nc.sync.dma_start(out=outT[nt * 128:(nt + 1) * 128, :], in_=ot)


def build():
    nc = bacc.Bacc("TRN2", target_bir_lowering=False, debug=False,
                   num_devices=NCORES)
    with tile.TileContext(nc) as tc:
        with (
            tc.tile_pool(name="const", bufs=1) as const,
            tc.tile_pool(name="persist", bufs=1) as persist,
            tc.tile_pool(name="dram", bufs=1, space="DRAM") as dram,
        ):
            _emit(nc, tc, {"const": const, "persist": persist, "dram": dram})
    nc.compile()
    return nc


_CACHED = {}


def _get_nc():
    if "nc" not in _CACHED:
        _CACHED["nc"] = build()
    return _CACHED["nc"]


def _prep_in_maps(inputs):
    gf = lambda k: np.asarray(inputs[k], np.float32)
    x = gf("x")
    tobf = lambda a: np.ascontiguousarray(a).astype(ml_dtypes.bfloat16)
    col = lambda a: np.ascontiguousarray(gf(a).reshape(-1, 1))
    tri01 = (np.arange(128)[:, None] > np.arange(128)[None, :]).astype(
        ml_dtypes.bfloat16)
    tri2_np = np.ascontiguousarray(np.stack([tri01, tri01], axis=1))
    shared = dict(wq=tobf(gf("wq")), wk=tobf(gf("wk")), wv=tobf(gf("wv")),
                  wo=tobf(gf("wo")), w1=tobf(gf("w1")), w2=tobf(gf("w2")),
                  bo_c=col("bo"), b1_c=col("b1"), b2_c=col("b2"),
                  g1_c=col("ln1_g"), bln1_c=col("ln1_b"),
                  g2_c=col("ln2_g"), bln2_c=col("ln2_b"), tri2=tri2_np)
    in_maps = []
    for c in range(NCORES):
        b, q = c // 4, c % 4
        m = dict(shared)
        m["xT"] = np.ascontiguousarray(x[b, q * R:(q + 1) * R, :].T)
        in_maps.append(m)
    return in_maps


def run(inputs, **kw):
    nc = _get_nc()
    in_maps = _prep_in_maps(inputs)
    res = bass_utils.run_bass_kernel_spmd(nc, in_maps,
                                          core_ids=list(range(NCORES)), **kw)
    out = np.empty((B, S, D), np.float32)
    for c in range(NCORES):
        b, q = c // 4, c % 4
        out[b, q * R:(q + 1) * R, :] = res.results[c]["outT"].T
    return out, res


def kernel(**inputs):
    out, _ = run(inputs)
    return out


# revision 9
# speedup vs baseline: 1.2288x; 1.0994x over previous
"""Trainium2 Bass kernel for a decoder layer (LN->attn->res, LN->attn->FFN->res).

Sharding (8 cores, SPMD single program, fully rank-uniform IR):
 - Row-parallel phases (LN / QKV / out-proj / FFN / residuals): global rows
   B*S = 4096 split 512/core: core c owns batch c//4, seq rows [512*(c%4), +512).
 - QKV computed data-parallel (each core: all 1024 features of its own 512
   rows), then redistributed head-parallel with three AllToAlls (k, q, v; each
   1 MB/core) that pipeline behind the projection matmuls. Attention core:
   head-parallel (2 heads/core over the full sequence, per batch).
 - Attention outputs return to row-parallel with one more AllToAll (1 MB/core)
   feeding a local out-projection. No AllGathers, no AllReduce: 4 collectives
   per attention with ~8x less wire than gather-based tensor parallelism.

Layout: activations feature-on-partition ([D, rows]); host pre-transposes x
and post-transposes the output. Matmuls bf16, accumulation fp32, residual fp32.

Masking: reference masks k <= q (strictly-upper attention). Softmax runs
without max-subtraction (scores bounded); diagonal 128x128 blocks are masked
multiplicatively AFTER exp with a 0/1 upper-strict mask on the vector engine
(exact zeros, no PE matmul); fully-masked tiles are skipped via per-tile
column prefixes. Softmax denominators ride in PSUM row 64 via a ones-column
appended to V. The all-masked last row (uniform attention over all 2048 keys)
is patched post-normalize with a DVE reduce of V over the full sequence.

LayerNorm stats are computed with a [128,128] ones stationary so every
per-row scalar op runs on all 128 lanes (no single-lane reciprocal/sqrt,
no DMA broadcast round-trip).
"""

import numpy as np
import ml_dtypes

import concourse.bass as bass
import concourse.bacc as bacc
import concourse.tile as tile
import concourse.mybir as mybir
from concourse import bass_utils

B, S, D, H, F = 2, 2048, 1024, 16, 4096
HD = D // H            # 64
NCORES = 8
R = S // 4             # 512 rows per core
KT16 = S // 128        # 16 seq tiles
DT8 = D // 128         # 8 feature tiles of D
FT32 = F // 128        # 32 feature tiles of F
EPS = 1e-5

f32 = mybir.dt.float32
bf16 = mybir.dt.bfloat16
GROUP8 = [list(range(8))]
AF = mybir.ActivationFunctionType
ALU = mybir.AluOpType


def _emit(nc, tc, ctxpools):
    # ---------------- I/O tensors ----------------
    xT = nc.dram_tensor("xT", [D, R], f32, kind="ExternalInput")
    wq = nc.dram_tensor("wq", [D, D], bf16, kind="ExternalInput")
    wk = nc.dram_tensor("wk", [D, D], bf16, kind="ExternalInput")
    wv = nc.dram_tensor("wv", [D, D], bf16, kind="ExternalInput")
    wo = nc.dram_tensor("wo", [D, D], bf16, kind="ExternalInput")
    w1 = nc.dram_tensor("w1", [D, F], bf16, kind="ExternalInput")
    w2 = nc.dram_tensor("w2", [F, D], bf16, kind="ExternalInput")
    bo_c = nc.dram_tensor("bo_c", [D, 1], f32, kind="ExternalInput")
    b1_c = nc.dram_tensor("b1_c", [F, 1], f32, kind="ExternalInput")
    b2_c = nc.dram_tensor("b2_c", [D, 1], f32, kind="ExternalInput")
    g1_c = nc.dram_tensor("g1_c", [D, 1], f32, kind="ExternalInput")
    bln1_c = nc.dram_tensor("bln1_c", [D, 1], f32, kind="ExternalInput")
    g2_c = nc.dram_tensor("g2_c", [D, 1], f32, kind="ExternalInput")
    bln2_c = nc.dram_tensor("bln2_c", [D, 1], f32, kind="ExternalInput")
    tri2 = nc.dram_tensor("tri2", [128, 2, 128], bf16, kind="ExternalInput")
    outT = nc.dram_tensor("outT", [D, R], f32, kind="ExternalOutput")

    const = ctxpools["const"]
    persist = ctxpools["persist"]
    dram = ctxpools["dram"]

    # ---------------- persistent activations ----------------
    resT = persist.tile([128, DT8, R], f32)    # residual stream (fp32)
    hT1 = persist.tile([128, DT8, R], bf16)    # LN output / attn input
    h2T = persist.tile([128, DT8, R], bf16)    # attn2 out -> FFN in
    OT = persist.tile([128, DT8, R], bf16)     # gathered attention out

    # x first: LN1 is the first consumer
    nc.sync.dma_start(out=resT, in_=xT[:, :].rearrange("(k p) q -> p k q", p=128))

    # ---------------- constants / weights resident in SBUF ----------------
    g1_sb = const.tile([128, DT8], f32)
    nc.sync.dma_start(out=g1_sb, in_=g1_c[:, 0:1].rearrange("(k p) o -> p (k o)", p=128))
    bln1_sb = const.tile([128, DT8], f32)
    nc.sync.dma_start(out=bln1_sb, in_=bln1_c[:, 0:1].rearrange("(k p) o -> p (k o)", p=128))
    g2_sb = const.tile([128, DT8], f32)
    nc.sync.dma_start(out=g2_sb, in_=g2_c[:, 0:1].rearrange("(k p) o -> p (k o)", p=128))
    bln2_sb = const.tile([128, DT8], f32)
    nc.sync.dma_start(out=bln2_sb, in_=bln2_c[:, 0:1].rearrange("(k p) o -> p (k o)", p=128))
    bo_sb = const.tile([128, DT8], f32)
    nc.sync.dma_start(out=bo_sb, in_=bo_c[:, 0:1].rearrange("(k p) o -> p (k o)", p=128))
    b1_sb = const.tile([128, FT32], f32)
    nc.sync.dma_start(out=b1_sb, in_=b1_c[:, 0:1].rearrange("(k p) o -> p (k o)", p=128))
    b2_sb = const.tile([128, DT8], f32)
    nc.sync.dma_start(out=b2_sb, in_=b2_c[:, 0:1].rearrange("(k p) o -> p (k o)", p=128))
    tri2_sb = const.tile([128, 2, 128], bf16)
    nc.sync.dma_start(out=tri2_sb, in_=tri2[:, :, :])
    ones128 = const.tile([128, 128], bf16)
    nc.vector.memset(ones128, 1.0)
    eps_sb = const.tile([128, 1], f32)
    nc.vector.memset(eps_sb, EPS)
    # QKV weights in projection order (k, q, v), then wo
    wk_sb = const.tile([128, DT8, D], bf16)
    nc.sync.dma_start(out=wk_sb, in_=wk[:, :].rearrange("(k p) n -> p k n", p=128))
    wq_sb = const.tile([128, DT8, D], bf16)
    nc.sync.dma_start(out=wq_sb, in_=wq[:, :].rearrange("(k p) n -> p k n", p=128))
    wv_sb = const.tile([128, DT8, D], bf16)
    nc.sync.dma_start(out=wv_sb, in_=wv[:, :].rearrange("(k p) n -> p k n", p=128))
    wo_sb = const.tile([128, DT8, D], bf16)
    nc.sync.dma_start(out=wo_sb, in_=wo[:, :].rearrange("(k p) n -> p k n", p=128))

    # =================================================================
    def layernorm(g_sb, bln_sb, hT):
        with (
            tc.tile_pool(name="ln_sb", bufs=2) as lnp,
            tc.tile_pool(name="ln_ps", bufs=1, space="PSUM") as lnps,
        ):
            ps_sum = lnps.tile([128, R], f32, tag="s")
            ps_sq = lnps.tile([128, R], f32, tag="q")
            for kt in range(DT8):
                rb = lnp.tile([128, R], bf16, tag="rb", bufs=3)
                nc.vector.tensor_copy(out=rb, in_=resT[:, kt, :])
                sq = lnp.tile([128, R], bf16, tag="sq", bufs=3)
                nc.vector.tensor_mul(out=sq, in0=rb, in1=rb)
                nc.tensor.matmul(ps_sum, ones128, rb,
                                 start=(kt == 0), stop=(kt == DT8 - 1))
                nc.tensor.matmul(ps_sq, ones128, sq,
                                 start=(kt == 0), stop=(kt == DT8 - 1))
            # all per-row scalars live on all 128 partitions (full DVE width)
            mu = lnp.tile([128, R], f32)
            nc.vector.tensor_scalar_mul(out=mu, in0=ps_sum, scalar1=1.0 / D)
            msq = lnp.tile([128, R], f32)
            nc.vector.tensor_scalar_mul(out=msq, in0=ps_sq, scalar1=1.0 / D)
            mu2 = lnp.tile([128, R], f32)
            nc.vector.tensor_mul(out=mu2, in0=mu, in1=mu)
            var = lnp.tile([128, R], f32)
            nc.vector.tensor_tensor(out=var, in0=msq, in1=mu2, op=ALU.subtract)
            sd = lnp.tile([128, R], f32)
            nc.scalar.activation(out=sd, in_=var, func=AF.Sqrt, bias=eps_sb)
            rstd = lnp.tile([128, R], f32)
            nc.vector.reciprocal(out=rstd, in_=sd)
            for kt in range(DT8):
                t1 = lnp.tile([128, R], f32, tag="t1", bufs=3)
                nc.vector.tensor_tensor(out=t1, in0=resT[:, kt, :],
                                        in1=mu, op=ALU.subtract)
                t2 = lnp.tile([128, R], f32, tag="t2", bufs=3)
                nc.vector.tensor_mul(out=t2, in0=t1, in1=rstd)
                nc.vector.tensor_scalar(out=hT[:, kt, :], in0=t2,
                                        scalar1=g_sb[:, kt:kt + 1],
                                        scalar2=bln_sb[:, kt:kt + 1],
                                        op0=ALU.mult, op1=ALU.add)

    # =================================================================
    def attention(hT, first):
        # -- data-parallel QKV over my 512 rows, all features; one AllToAll
        #    (slot d carries my k/q/v features of head-group d) redistributes
        #    to head-parallel.
        ain = dram.tile([3 * D, R], bf16, tag="a2ai", bufs=2)
        aout = dram.tile([3 * D, R], bf16, tag="a2ao", bufs=2)
        ain_v = ain.rearrange("(n t p) q -> t p n q", t=3, p=128)
        aout_v = aout.rearrange("(n t p) q -> t n p q", t=3, p=128)
        with (
            tc.tile_pool(name="qkv_sb", bufs=2) as qsb,
            tc.tile_pool(name="qkv_ps", bufs=4, space="PSUM") as qps,
        ):
            for p, (wsb, scale) in enumerate(((wk_sb, None),
                                              (wq_sb, 0.125),
                                              (wv_sb, None))):
                st = qsb.tile([128, DT8, R], bf16, tag="st", name=f"st{p}")
                for nt in range(DT8):
                    ps = qps.tile([128, R], f32, tag="ps")
                    for kt in range(DT8):
                        nc.tensor.matmul(ps, wsb[:, kt, nt * 128:nt * 128 + 128],
                                         hT[:, kt, :],
                                         start=(kt == 0), stop=(kt == DT8 - 1))
                    if scale is None:
                        nc.vector.tensor_copy(out=st[:, nt, :], in_=ps)
                    else:
                        nc.vector.tensor_scalar_mul(out=st[:, nt, :], in0=ps,
                                                    scalar1=scale)
                nc.sync.dma_start(out=ain_v[p], in_=st)
        nc.gpsimd.collective_compute(
            "AllToAll", ALU.bypass, replica_groups=GROUP8,
            ins=[ain.opt()], outs=[aout.opt()])

        # -- attention core: my 2 heads, both batches over full sequence
        a2a_o_in = dram.tile([NCORES * 128, R], bf16, tag="a2aoi", bufs=2)
        a2a_o_out = dram.tile([NCORES * 128, R], bf16, tag="a2aoo", bufs=2)
        with tc.tile_pool(name="at_sb", bufs=2) as ap:
            for b in range(B):
                qT = ap.tile([128, 4, R], bf16, tag="qT")
                kT = ap.tile([128, 4, R], bf16, tag="kT")
                vT = ap.tile([128, 4, R], bf16, tag="vT")
                for p, dst in ((0, kT), (1, qT), (2, vT)):
                    nc.sync.dma_start(
                        out=dst,
                        in_=aout_v[p, 4 * b:4 * b + 4].rearrange(
                            "c p q -> p c q"))

                # V transpose -> v_aug [128(seq), kt, h, 65]
                vaug = ap.tile([128, KT16, 2, HD + 1], bf16, tag="vaug")
                nc.vector.memset(vaug[:, :, :, HD:HD + 1], 1.0)
                for kt in range(KT16):
                    vtmp = ap.tile([128, 128], bf16, tag="vtmp", bufs=4)
                    nc.sync.dma_start(
                        out=vtmp,
                        in_=vT[:, kt // 4, (kt % 4) * 128:(kt % 4) * 128 + 128],
                        transpose=True)
                    nc.vector.tensor_copy(out=vaug[:, kt, 0, 0:HD],
                                          in_=vtmp[:, 0:HD])
                    nc.vector.tensor_copy(out=vaug[:, kt, 1, 0:HD],
                                          in_=vtmp[:, HD:128])
                # mean of V over the whole sequence (for the all-masked last row)
                sumv = ap.tile([128, 1], f32, tag="sumv")
                nc.vector.tensor_reduce(out=sumv, in_=vT,
                                        axis=mybir.AxisListType.XY, op=ALU.add)

                oT = ap.tile([128, 4, R], bf16, tag="oT")
                with (
                    tc.tile_pool(name="sc_ps", bufs=2, space="PSUM") as scps,
                    tc.tile_pool(name="av_ps", bufs=4, space="PSUM") as avps,
                ):
                    for c in range(4):
                        po = [avps.tile([128, R], f32, tag="po", name=f"po{hh}")
                              for hh in range(2)]
                        for kt in range(KT16 - 1, 4 * c - 1, -1):
                            npfx = min(kt - 4 * c + 1, 4) * 128
                            pss = scps.tile([128, 2, R], f32, tag="pss")
                            for hh in range(2):
                                lo = hh * HD
                                nc.tensor.matmul(
                                    pss[:, hh, 0:npfx],
                                    kT[lo:lo + HD, kt // 4,
                                       (kt % 4) * 128:(kt % 4) * 128 + 128],
                                    qT[lo:lo + HD, c, 0:npfx],
                                    start=True, stop=True)
                            wt = ap.tile([128, 2, R], bf16, tag="wt", bufs=3)
                            nc.scalar.activation(out=wt[:, :, 0:npfx],
                                                 in_=pss[:, :, 0:npfx],
                                                 func=AF.Exp)
                            if kt <= 4 * c + 3:
                                # diagonal block: exact multiplicative mask
                                nc.vector.tensor_mul(
                                    out=wt[:, :, npfx - 128:npfx],
                                    in0=wt[:, :, npfx - 128:npfx],
                                    in1=tri2_sb)
                            for hh in range(2):
                                nc.tensor.matmul(
                                    po[hh][0:HD + 1, 0:npfx],
                                    vaug[:, kt, hh, :],
                                    wt[:, hh, 0:npfx],
                                    start=(kt == KT16 - 1),
                                    stop=(kt == 4 * c))
                        # normalize by PSUM row HD (softmax denominators).
                        # Reciprocal is ~6.4ns/elem along the free dim, so
                        # reshape the 1024 denominators to [128, 8] (DMA
                        # round-trip) before inverting, then broadcast the
                        # reciprocals.
                        den_sb = ap.tile([1, 2, R], f32, tag="densb", bufs=4)
                        for hh in range(2):
                            nc.vector.tensor_copy(out=den_sb[:, hh, :],
                                                  in_=po[hh][HD:HD + 1, :])
                        scr2 = dram.tile([1, 2, R], f32, tag="rscr", bufs=4)
                        nc.sync.dma_start(out=scr2, in_=den_sb)
                        denp = ap.tile([128, 8], f32, tag="denp", bufs=4)
                        nc.sync.dma_start(
                            out=denp,
                            in_=scr2[0].rearrange("h (p f) -> (h p) f", p=64))
                        recp = ap.tile([128, 8], f32, tag="recp", bufs=4)
                        nc.vector.reciprocal(out=recp, in_=denp)
                        scr3 = dram.tile([1, 2, R], f32, tag="rscr3", bufs=4)
                        nc.sync.dma_start(
                            out=scr3[0].rearrange("h (p f) -> (h p) f", p=64),
                            in_=recp)
                        recb = ap.tile([HD, 2, R], f32, tag="recb", bufs=2)
                        nc.sync.dma_start(
                            out=recb,
                            in_=scr3[0][None].broadcast_to([HD, 2, R]))
                        for hh in range(2):
                            nc.vector.tensor_mul(
                                out=oT[hh * HD:hh * HD + HD, c, :],
                                in0=po[hh][0:HD, :], in1=recb[:, hh, :])
                        if c == 3:
                            # all-masked last row: uniform attention = mean(V)
                            nc.vector.tensor_scalar_mul(
                                out=oT[:, 3, R - 1:R], in0=sumv,
                                scalar1=1.0 / S)
                nc.sync.dma_start(
                    out=a2a_o_in[bass.ds(4 * b * 128, 512), :].rearrange(
                        "(c p) q -> p c q", c=4),
                    in_=oT)
        nc.gpsimd.collective_compute(
            "AllToAll", ALU.bypass, replica_groups=GROUP8,
            ins=[a2a_o_in.opt()], outs=[a2a_o_out.opt()])
        nc.sync.dma_start(
            out=OT, in_=a2a_o_out[:, :].rearrange("(s p) q -> p s q", p=128))

        # ---- out-projection (+ bias, + residual or -> h2T) ----
        with (
            tc.tile_pool(name="op_sb", bufs=3) as opp,
            tc.tile_pool(name="op_ps", bufs=4, space="PSUM") as opps,
        ):
            for nt in range(DT8):
                ps = opps.tile([128, R], f32, tag="ps")
                for s in range(DT8):
                    nc.tensor.matmul(ps, wo_sb[:, s, nt * 128:nt * 128 + 128],
                                     OT[:, s, :],
                                     start=(s == 0), stop=(s == DT8 - 1))
                if first:
                    t = opp.tile([128, R], f32, tag="t")
                    nc.vector.tensor_scalar(out=t, in0=ps,
                                            scalar1=bo_sb[:, nt:nt + 1],
                                            scalar2=None, op0=ALU.add)
                    nc.vector.tensor_add(out=resT[:, nt, :],
                                         in0=resT[:, nt, :], in1=t)
                else:
                    nc.vector.tensor_scalar(out=h2T[:, nt, :], in0=ps,
                                            scalar1=bo_sb[:, nt:nt + 1],
                                            scalar2=None, op0=ALU.add)

    # =================================================================
    # layer body
    layernorm(g1_sb, bln1_sb, hT1)
    attention(hT1, first=True)
    layernorm(g2_sb, bln2_sb, hT1)
    attention(hT1, first=False)

    # ---- FFN ----
    with (
        tc.tile_pool(name="ffn_sb", bufs=3) as fp,
        tc.tile_pool(name="ffn_big", bufs=1) as fbig,
        tc.tile_pool(name="ffn_ps", bufs=4, space="PSUM") as fps,
    ):
        gT = fbig.tile([128, FT32, R], bf16)
        w1v = w1[:, :].rearrange("(k p) (nt n) -> nt k p n", p=128, n=128)
        for nt in range(FT32):
            w1t = fp.tile([128, DT8, 128], bf16, tag="w1t", bufs=4)
            nc.sync.dma_start(out=w1t, in_=w1v[nt].rearrange("k p n -> p k n"))
            ps = fps.tile([128, R], f32, tag="ps1")
            for kt in range(DT8):
                nc.tensor.matmul(ps, w1t[:, kt, :], h2T[:, kt, :],
                                 start=(kt == 0), stop=(kt == DT8 - 1))
            nc.scalar.activation(out=gT[:, nt, :], in_=ps, func=AF.Gelu,
                                 bias=b1_sb[:, nt:nt + 1])
        w2v = w2[:, :].rearrange("(k p) (nt n) -> nt k p n", p=128, n=128)
        for nt in range(DT8):
            w2t = fp.tile([128, FT32, 128], bf16, tag="w2t", bufs=3)
            nc.sync.dma_start(out=w2t, in_=w2v[nt].rearrange("k p n -> p k n"))
            ps2 = fps.tile([128, R], f32, tag="ps2")
            for kt in range(FT32):
                nc.tensor.matmul(ps2, w2t[:, kt, :], gT[:, kt, :],
                                 start=(kt == 0), stop=(kt == FT32 - 1))
            t = fp.tile([128, R], f32, tag="t")
            nc.vector.tensor_scalar(out=t, in0=ps2,
                                    scalar1=b2_sb[:, nt:nt + 1],
                                    scalar2=None, op0=ALU.add)
            ot = fp.tile([128, R], f32, tag="ot")
            nc.vector.tensor_add(out=ot, in0=t, in1=resT[:, nt, :])
            nc.sync.dma_start(out=outT[nt * 128:(nt + 1) * 128, :], in_=ot)


def build():
    nc = bacc.Bacc("TRN2", target_bir_lowering=False, debug=False,
                   num_devices=NCORES)
    with tile.TileContext(nc) as tc:
        with (
            tc.tile_pool(name="const", bufs=1) as const,
            tc.tile_pool(name="persist", bufs=1) as persist,
            tc.tile_pool(name="dram", bufs=1, space="DRAM") as dram,
        ):
            _emit(nc, tc, {"const": const, "persist": persist, "dram": dram})
    nc.compile()
    return nc


_CACHED = {}


def _get_nc():
    if "nc" not in _CACHED:
        _CACHED["nc"] = build()
    return _CACHED["nc"]


def _prep_in_maps(inputs):
    gf = lambda k: np.asarray(inputs[k], np.float32)
    x = gf("x")
    tobf = lambda a: np.ascontiguousarray(a).astype(ml_dtypes.bfloat16)
    col = lambda a: np.ascontiguousarray(gf(a).reshape(-1, 1))
    tri01 = (np.arange(128)[:, None] > np.arange(128)[None, :]).astype(
        ml_dtypes.bfloat16)
    tri2_np = np.ascontiguousarray(np.stack([tri01, tri01], axis=1))
    shared = dict(wq=tobf(gf("wq")), wk=tobf(gf("wk")), wv=tobf(gf("wv")),
                  wo=tobf(gf("wo")), w1=tobf(gf("w1")), w2=tobf(gf("w2")),
                  bo_c=col("bo"), b1_c=col("b1"), b2_c=col("b2"),
                  g1_c=col("ln1_g"), bln1_c=col("ln1_b"),
                  g2_c=col("ln2_g"), bln2_c=col("ln2_b"), tri2=tri2_np)
    in_maps = []
    for c in range(NCORES):
        b, q = c // 4, c % 4
        m = dict(shared)
        m["xT"] = np.ascontiguousarray(x[b, q * R:(q + 1) * R, :].T)
        in_maps.append(m)
    return in_maps


def run(inputs, **kw):
    nc = _get_nc()
    in_maps = _prep_in_maps(inputs)
    res = bass_utils.run_bass_kernel_spmd(nc, in_maps,
                                          core_ids=list(range(NCORES)), **kw)
    out = np.empty((B, S, D), np.float32)
    for c in range(NCORES):
        b, q = c // 4, c % 4
        out[b, q * R:(q + 1) * R, :] = res.results[c]["outT"].T
    return out, res


def kernel(**inputs):
    out, _ = run(inputs)
    return out


# revision 10
# speedup vs baseline: 1.2575x; 1.0233x over previous
"""Trainium2 Bass kernel for a decoder layer (LN->attn->res, LN->attn->FFN->res).

Sharding (8 cores, SPMD single program):
 - Row-parallel phases (LN / out-proj / FFN / residuals): global rows B*S =
   4096 split 512/core: core c owns batch c//4, seq rows [512*(c%4), +512).
 - Attention head-parallel: core c computes 2 global heads {2c, 2c+1} for
   both batches over the full sequence (per-core column-sliced QKV weights).
 - Collectives: one 8-core AllGather of the LN output per attention (the
   cheap direction: 1 MB in / 7 MB received), and one AllToAll of the
   attention outputs (1 MB, ~8x less wire than the AllGather equivalent)
   feeding a local out-projection.
 - Dummy PE matmuls are streamed during both collective windows to keep the
   HAM clock gate at 2.4 GHz (otherwise the post-collective matmuls run at
   1.2 GHz for ~3.4 us).

Layout: activations feature-on-partition ([D, rows]); host pre-transposes x
and post-transposes the output. Matmuls bf16, accumulation fp32, residual fp32.

Masking: reference masks k <= q (strictly-upper attention). Softmax runs
without max-subtraction (scores bounded); both heads' score tiles share one
2-bank PSUM tile so a single Exp covers them; diagonal 128x128 blocks are
masked multiplicatively AFTER exp with a 0/1 upper-strict mask on the vector
engine; fully-masked tiles are skipped via per-tile column prefixes. Softmax
denominators ride in PSUM row 64 via a ones-column appended to V; they are
reshaped to [128,8] through DRAM before the reciprocal (DVE reciprocal is
~6.4ns/elem along the free dim). The all-masked last row (uniform attention
over all 2048 keys) is patched post-normalize with a DVE reduce of V.

LayerNorm stats use a [128,128] ones stationary so every per-row scalar op
runs on all 128 lanes.
"""

import numpy as np
import ml_dtypes

import concourse.bass as bass
import concourse.bacc as bacc
import concourse.tile as tile
import concourse.mybir as mybir
from concourse import bass_utils

B, S, D, H, F = 2, 2048, 1024, 16, 4096
HD = D // H            # 64
NCORES = 8
R = S // 4             # 512 rows per core
KT16 = S // 128        # 16 seq tiles
DT8 = D // 128         # 8 feature tiles of D
FT32 = F // 128        # 32 feature tiles of F
EPS = 1e-5

f32 = mybir.dt.float32
bf16 = mybir.dt.bfloat16
GROUP8 = [list(range(8))]
AF = mybir.ActivationFunctionType
ALU = mybir.AluOpType


def _emit(nc, tc, ctxpools):
    # ---------------- I/O tensors ----------------
    xT = nc.dram_tensor("xT", [D, R], f32, kind="ExternalInput")
    wq_my = nc.dram_tensor("wq_my", [D, 2 * HD], bf16, kind="ExternalInput")
    wk_my = nc.dram_tensor("wk_my", [D, 2 * HD], bf16, kind="ExternalInput")
    wv_my = nc.dram_tensor("wv_my", [D, 2 * HD], bf16, kind="ExternalInput")
    wo = nc.dram_tensor("wo", [D, D], bf16, kind="ExternalInput")
    w1 = nc.dram_tensor("w1", [D, F], bf16, kind="ExternalInput")
    w2 = nc.dram_tensor("w2", [F, D], bf16, kind="ExternalInput")
    bo_c = nc.dram_tensor("bo_c", [D, 1], f32, kind="ExternalInput")
    b1_c = nc.dram_tensor("b1_c", [F, 1], f32, kind="ExternalInput")
    b2_c = nc.dram_tensor("b2_c", [D, 1], f32, kind="ExternalInput")
    g1_c = nc.dram_tensor("g1_c", [D, 1], f32, kind="ExternalInput")
    bln1_c = nc.dram_tensor("bln1_c", [D, 1], f32, kind="ExternalInput")
    g2_c = nc.dram_tensor("g2_c", [D, 1], f32, kind="ExternalInput")
    bln2_c = nc.dram_tensor("bln2_c", [D, 1], f32, kind="ExternalInput")
    tri2 = nc.dram_tensor("tri2", [128, 2, 128], bf16, kind="ExternalInput")
    outT = nc.dram_tensor("outT", [D, R], f32, kind="ExternalOutput")

    const = ctxpools["const"]
    persist = ctxpools["persist"]
    dram = ctxpools["dram"]

    # ---------------- persistent activations ----------------
    resT = persist.tile([128, DT8, R], f32)    # residual stream (fp32)
    hT1 = persist.tile([128, DT8, R], bf16)    # LN output / attn input
    h2T = persist.tile([128, DT8, R], bf16)    # attn2 out -> FFN in
    OT = persist.tile([128, DT8, R], bf16)     # gathered attention out

    # x first: LN1 is the first consumer
    nc.sync.dma_start(out=resT, in_=xT[:, :].rearrange("(k p) q -> p k q", p=128))

    # ---------------- constants / weights resident in SBUF ----------------
    g1_sb = const.tile([128, DT8], f32)
    nc.sync.dma_start(out=g1_sb, in_=g1_c[:, 0:1].rearrange("(k p) o -> p (k o)", p=128))
    bln1_sb = const.tile([128, DT8], f32)
    nc.sync.dma_start(out=bln1_sb, in_=bln1_c[:, 0:1].rearrange("(k p) o -> p (k o)", p=128))
    g2_sb = const.tile([128, DT8], f32)
    nc.sync.dma_start(out=g2_sb, in_=g2_c[:, 0:1].rearrange("(k p) o -> p (k o)", p=128))
    bln2_sb = const.tile([128, DT8], f32)
    nc.sync.dma_start(out=bln2_sb, in_=bln2_c[:, 0:1].rearrange("(k p) o -> p (k o)", p=128))
    bo_sb = const.tile([128, DT8], f32)
    nc.sync.dma_start(out=bo_sb, in_=bo_c[:, 0:1].rearrange("(k p) o -> p (k o)", p=128))
    b1_sb = const.tile([128, FT32], f32)
    nc.sync.dma_start(out=b1_sb, in_=b1_c[:, 0:1].rearrange("(k p) o -> p (k o)", p=128))
    b2_sb = const.tile([128, DT8], f32)
    nc.sync.dma_start(out=b2_sb, in_=b2_c[:, 0:1].rearrange("(k p) o -> p (k o)", p=128))
    tri2_sb = const.tile([128, 2, 128], bf16)
    nc.sync.dma_start(out=tri2_sb, in_=tri2[:, :, :])
    ones128 = const.tile([128, 128], bf16)
    nc.vector.memset(ones128, 1.0)
    eps_sb = const.tile([128, 1], f32)
    nc.vector.memset(eps_sb, EPS)
    wrm = const.tile([128, R], bf16)
    nc.vector.memset(wrm, 0.25)
    wq_sb = const.tile([128, DT8, 2 * HD], bf16)
    nc.sync.dma_start(out=wq_sb, in_=wq_my[:, :].rearrange("(k p) n -> p k n", p=128))
    wk_sb = const.tile([128, DT8, 2 * HD], bf16)
    nc.sync.dma_start(out=wk_sb, in_=wk_my[:, :].rearrange("(k p) n -> p k n", p=128))
    wv_sb = const.tile([128, DT8, 2 * HD], bf16)
    nc.sync.dma_start(out=wv_sb, in_=wv_my[:, :].rearrange("(k p) n -> p k n", p=128))
    wo_sb = const.tile([128, DT8, D], bf16)
    nc.sync.dma_start(out=wo_sb, in_=wo[:, :].rearrange("(k p) n -> p k n", p=128))

    warm_ctr = [0]

    def warm_pe(n):
        # dead matmuls that keep the HAM clock gate open through a
        # collective window; sunk into a DRAM scratch to survive DCE
        warm_ctr[0] += 1
        with (
            tc.tile_pool(name=f"warm{warm_ctr[0]}", bufs=1) as wp,
            tc.tile_pool(name=f"warm_ps{warm_ctr[0]}", bufs=1,
                         space="PSUM") as wps,
        ):
            psw = wps.tile([128, R], f32)
            for _ in range(n):
                nc.tensor.matmul(psw, ones128, wrm, start=True, stop=True)
            sink = wp.tile([128, R], bf16)
            nc.vector.tensor_copy(out=sink, in_=psw)
            scr = dram.tile([128, R], bf16, tag="warmscr", bufs=2)
            nc.sync.dma_start(out=scr, in_=sink)

    # =================================================================
    def layernorm(g_sb, bln_sb, hT):
        with (
            tc.tile_pool(name="ln_sb", bufs=2) as lnp,
            tc.tile_pool(name="ln_ps", bufs=1, space="PSUM") as lnps,
        ):
            ps_sum = lnps.tile([128, R], f32, tag="s")
            ps_sq = lnps.tile([128, R], f32, tag="q")
            for kt in range(DT8):
                rb = lnp.tile([128, R], bf16, tag="rb", bufs=3)
                nc.vector.tensor_copy(out=rb, in_=resT[:, kt, :])
                sq = lnp.tile([128, R], bf16, tag="sq", bufs=3)
                nc.vector.tensor_mul(out=sq, in0=rb, in1=rb)
                nc.tensor.matmul(ps_sum, ones128, rb,
                                 start=(kt == 0), stop=(kt == DT8 - 1))
                nc.tensor.matmul(ps_sq, ones128, sq,
                                 start=(kt == 0), stop=(kt == DT8 - 1))
            # all per-row scalars live on all 128 partitions (full DVE width)
            mu = lnp.tile([128, R], f32)
            nc.vector.tensor_scalar_mul(out=mu, in0=ps_sum, scalar1=1.0 / D)
            msq = lnp.tile([128, R], f32)
            nc.vector.tensor_scalar_mul(out=msq, in0=ps_sq, scalar1=1.0 / D)
            mu2 = lnp.tile([128, R], f32)
            nc.vector.tensor_mul(out=mu2, in0=mu, in1=mu)
            var = lnp.tile([128, R], f32)
            nc.vector.tensor_tensor(out=var, in0=msq, in1=mu2, op=ALU.subtract)
            sd = lnp.tile([128, R], f32)
            nc.scalar.activation(out=sd, in_=var, func=AF.Sqrt, bias=eps_sb)
            rstd = lnp.tile([128, R], f32)
            nc.vector.reciprocal(out=rstd, in_=sd)
            for kt in range(DT8):
                t1 = lnp.tile([128, R], f32, tag="t1", bufs=3)
                nc.vector.tensor_tensor(out=t1, in0=resT[:, kt, :],
                                        in1=mu, op=ALU.subtract)
                t2 = lnp.tile([128, R], f32, tag="t2", bufs=3)
                nc.vector.tensor_mul(out=t2, in0=t1, in1=rstd)
                nc.vector.tensor_scalar(out=hT[:, kt, :], in0=t2,
                                        scalar1=g_sb[:, kt:kt + 1],
                                        scalar2=bln_sb[:, kt:kt + 1],
                                        op0=ALU.mult, op1=ALU.add)

    # =================================================================
    def attention(hT, first):
        # AllGather the LN output; each core then computes q/k/v for its own
        # 2 heads over the full sequence from host-sliced weight columns.
        ag_in = dram.tile([D, R], bf16, tag="ag1i", bufs=2)
        ag_out = dram.tile([NCORES * D, R], bf16, addr_space="Shared",
                           tag="ag1o", bufs=2)
        nc.sync.dma_start(
            out=ag_in.rearrange("(k p) q -> p k q", p=128), in_=hT)
        nc.gpsimd.collective_compute(
            "AllGather", ALU.bypass, replica_groups=GROUP8,
            ins=[ag_in.opt()], outs=[ag_out.opt()])
        ag1v = ag_out.rearrange("(r k p) q -> r k p q", r=NCORES, k=DT8, p=128)
        warm_pe(110)

        a2a_o_in = dram.tile([NCORES * 128, R], bf16, tag="a2aoi", bufs=2)
        a2a_o_out = dram.tile([NCORES * 128, R], bf16, tag="a2aoo", bufs=2)
        with (
            tc.tile_pool(name="at_sb", bufs=2) as ap,
            tc.tile_pool(name="at_big", bufs=2) as bigp,
        ):
            for b in range(B):
                # ---- gather LN output for batch b: [128, kt, 2048] ----
                hfull = bigp.tile([128, DT8, S], bf16, tag="hfull")
                for kt in range(DT8):
                    nc.sync.dma_start(
                        out=hfull[:, kt, :].rearrange("p (r q) -> p r q", r=4),
                        in_=ag1v[4 * b:4 * b + 4, kt].rearrange(
                            "r p q -> p r q"))

                # ---- QKV for my 2 heads, full sequence ----
                qT = ap.tile([128, 4, R], bf16, tag="qT")
                kT = ap.tile([128, 4, R], bf16, tag="kT")
                vT = ap.tile([128, 4, R], bf16, tag="vT")
                with tc.tile_pool(name="qkv_ps", bufs=4, space="PSUM") as qps:
                    for dst, wsb, scale in ((kT, wk_sb, None),
                                            (qT, wq_sb, 0.125),
                                            (vT, wv_sb, None)):
                        for c in range(4):
                            ps = qps.tile([128, R], f32, tag="ps")
                            for kt in range(DT8):
                                nc.tensor.matmul(
                                    ps, wsb[:, kt, :],
                                    hfull[:, kt, c * R:(c + 1) * R],
                                    start=(kt == 0), stop=(kt == DT8 - 1))
                            if scale is None:
                                nc.vector.tensor_copy(out=dst[:, c, :], in_=ps)
                            else:
                                nc.vector.tensor_scalar_mul(
                                    out=dst[:, c, :], in0=ps, scalar1=scale)

                # ---- V transpose -> v_aug [128(seq), kt, h, 65] ----
                vaug = ap.tile([128, KT16, 2, HD + 1], bf16, tag="vaug")
                nc.vector.memset(vaug[:, :, :, HD:HD + 1], 1.0)
                for kt in range(KT16):
                    vtmp = ap.tile([128, 128], bf16, tag="vtmp", bufs=4)
                    nc.sync.dma_start(
                        out=vtmp,
                        in_=vT[:, kt // 4, (kt % 4) * 128:(kt % 4) * 128 + 128],
                        transpose=True)
                    nc.vector.tensor_copy(out=vaug[:, kt, 0, 0:HD],
                                          in_=vtmp[:, 0:HD])
                    nc.vector.tensor_copy(out=vaug[:, kt, 1, 0:HD],
                                          in_=vtmp[:, HD:128])
                # mean of V over the whole sequence (all-masked last row)
                sumv = ap.tile([128, 1], f32, tag="sumv")
                nc.vector.tensor_reduce(out=sumv, in_=vT,
                                        axis=mybir.AxisListType.XY, op=ALU.add)

                oT = ap.tile([128, 4, R], bf16, tag="oT")
                with (
                    tc.tile_pool(name="sc_ps", bufs=2, space="PSUM") as scps,
                    tc.tile_pool(name="av_ps", bufs=4, space="PSUM") as avps,
                ):
                    for c in range(4):
                        po = [avps.tile([128, R], f32, tag="po", name=f"po{hh}")
                              for hh in range(2)]
                        for kt in range(KT16 - 1, 4 * c - 1, -1):
                            npfx = min(kt - 4 * c + 1, 4) * 128
                            pss = scps.tile([128, 2, R], f32, tag="pss")
                            for hh in range(2):
                                lo = hh * HD
                                nc.tensor.matmul(
                                    pss[:, hh, 0:npfx],
                                    kT[lo:lo + HD, kt // 4,
                                       (kt % 4) * 128:(kt % 4) * 128 + 128],
                                    qT[lo:lo + HD, c, 0:npfx],
                                    start=True, stop=True)
                            wt = ap.tile([128, 2, R], bf16, tag="wt", bufs=3)
                            nc.scalar.activation(out=wt[:, :, 0:npfx],
                                                 in_=pss[:, :, 0:npfx],
                                                 func=AF.Exp)
                            if kt <= 4 * c + 3:
                                # diagonal block: exact multiplicative mask
                                nc.vector.tensor_mul(
                                    out=wt[:, :, npfx - 128:npfx],
                                    in0=wt[:, :, npfx - 128:npfx],
                                    in1=tri2_sb)
                            for hh in range(2):
                                nc.tensor.matmul(
                                    po[hh][0:HD + 1, 0:npfx],
                                    vaug[:, kt, hh, :],
                                    wt[:, hh, 0:npfx],
                                    start=(kt == KT16 - 1),
                                    stop=(kt == 4 * c))
                        # normalize by PSUM row HD (softmax denominators);
                        # reshape the 1024 denominators to [128, 8] through
                        # DRAM before inverting (DVE reciprocal is
                        # ~6.4ns/elem along the free dim), then broadcast.
                        den_sb = ap.tile([1, 2, R], f32, tag="densb", bufs=4)
                        for hh in range(2):
                            nc.vector.tensor_copy(out=den_sb[:, hh, :],
                                                  in_=po[hh][HD:HD + 1, :])
                        scr2 = dram.tile([1, 2, R], f32, tag="rscr", bufs=4)
                        nc.sync.dma_start(out=scr2, in_=den_sb)
                        denp = ap.tile([128, 8], f32, tag="denp", bufs=4)
                        nc.sync.dma_start(
                            out=denp,
                            in_=scr2[0].rearrange("h (p f) -> (h p) f", p=64))
                        recp = ap.tile([128, 8], f32, tag="recp", bufs=4)
                        nc.vector.reciprocal(out=recp, in_=denp)
                        scr3 = dram.tile([1, 2, R], f32, tag="rscr3", bufs=4)
                        nc.sync.dma_start(
                            out=scr3[0].rearrange("h (p f) -> (h p) f", p=64),
                            in_=recp)
                        recb = ap.tile([HD, 2, R], f32, tag="recb", bufs=2)
                        nc.sync.dma_start(
                            out=recb,
                            in_=scr3[0][None].broadcast_to([HD, 2, R]))
                        for hh in range(2):
                            nc.vector.tensor_mul(
                                out=oT[hh * HD:hh * HD + HD, c, :],
                                in0=po[hh][0:HD, :], in1=recb[:, hh, :])
                        if c == 3:
                            # all-masked last row: uniform attention = mean(V)
                            nc.vector.tensor_scalar_mul(
                                out=oT[:, 3, R - 1:R], in0=sumv,
                                scalar1=1.0 / S)
                nc.sync.dma_start(
                    out=a2a_o_in[bass.ds(4 * b * 128, 512), :].rearrange(
                        "(c p) q -> p c q", c=4),
                    in_=oT)
        nc.gpsimd.collective_compute(
            "AllToAll", ALU.bypass, replica_groups=GROUP8,
            ins=[a2a_o_in.opt()], outs=[a2a_o_out.opt()])
        warm_pe(70)
        nc.sync.dma_start(
            out=OT, in_=a2a_o_out[:, :].rearrange("(s p) q -> p s q", p=128))

        # ---- out-projection (+ bias, + residual or -> h2T) ----
        with (
            tc.tile_pool(name="op_sb", bufs=3) as opp,
            tc.tile_pool(name="op_ps", bufs=4, space="PSUM") as opps,
        ):
            for nt in range(DT8):
                ps = opps.tile([128, R], f32, tag="ps")
                for s in range(DT8):
                    nc.tensor.matmul(ps, wo_sb[:, s, nt * 128:nt * 128 + 128],
                                     OT[:, s, :],
                                     start=(s == 0), stop=(s == DT8 - 1))
                if first:
                    t = opp.tile([128, R], f32, tag="t")
                    nc.vector.tensor_scalar(out=t, in0=ps,
                                            scalar1=bo_sb[:, nt:nt + 1],
                                            scalar2=None, op0=ALU.add)
                    nc.vector.tensor_add(out=resT[:, nt, :],
                                         in0=resT[:, nt, :], in1=t)
                else:
                    nc.vector.tensor_scalar(out=h2T[:, nt, :], in0=ps,
                                            scalar1=bo_sb[:, nt:nt + 1],
                                            scalar2=None, op0=ALU.add)

    # =================================================================
    # layer body
    layernorm(g1_sb, bln1_sb, hT1)
    attention(hT1, first=True)
    layernorm(g2_sb, bln2_sb, hT1)
    attention(hT1, first=False)

    # ---- FFN ----
    with (
        tc.tile_pool(name="ffn_sb", bufs=3) as fp,
        tc.tile_pool(name="ffn_big", bufs=1) as fbig,
        tc.tile_pool(name="ffn_ps", bufs=4, space="PSUM") as fps,
    ):
        gT = fbig.tile([128, FT32, R], bf16)
        w1v = w1[:, :].rearrange("(k p) (nt n) -> nt k p n", p=128, n=128)
        for nt in range(FT32):
            w1t = fp.tile([128, DT8, 128], bf16, tag="w1t", bufs=4)
            nc.sync.dma_start(out=w1t, in_=w1v[nt].rearrange("k p n -> p k n"))
            ps = fps.tile([128, R], f32, tag="ps1")
            for kt in range(DT8):
                nc.tensor.matmul(ps, w1t[:, kt, :], h2T[:, kt, :],
                                 start=(kt == 0), stop=(kt == DT8 - 1))
            nc.scalar.activation(out=gT[:, nt, :], in_=ps, func=AF.Gelu,
                                 bias=b1_sb[:, nt:nt + 1])
        w2v = w2[:, :].rearrange("(k p) (nt n) -> nt k p n", p=128, n=128)
        for nt in range(DT8):
            w2t = fp.tile([128, FT32, 128], bf16, tag="w2t", bufs=3)
            nc.sync.dma_start(out=w2t, in_=w2v[nt].rearrange("k p n -> p k n"))
            ps2 = fps.tile([128, R], f32, tag="ps2")
            for kt in range(FT32):
                nc.tensor.matmul(ps2, w2t[:, kt, :], gT[:, kt, :],
                                 start=(kt == 0), stop=(kt == FT32 - 1))
            t = fp.tile([128, R], f32, tag="t")
            nc.vector.tensor_scalar(out=t, in0=ps2,
                                    scalar1=b2_sb[:, nt:nt + 1],
                                    scalar2=None, op0=ALU.add)
            ot = fp.tile([128, R], f32, tag="ot")
            nc.vector.tensor_add(out=ot, in0=t, in1=resT[:, nt, :])
            nc.sync.dma_start(out=outT[nt * 128:(nt + 1) * 128, :], in_=ot)


def build():
    nc = bacc.Bacc("TRN2", target_bir_lowering=False, debug=False,
                   num_devices=NCORES)
    with tile.TileContext(nc) as tc:
        with (
            tc.tile_pool(name="const", bufs=1) as const,
            tc.tile_pool(name="persist", bufs=1) as persist,
            tc.tile_pool(name="dram", bufs=1, space="DRAM") as dram,
        ):
            _emit(nc, tc, {"const": const, "persist": persist, "dram": dram})
    nc.compile()
    return nc


_CACHED = {}


def _get_nc():
    if "nc" not in _CACHED:
        _CACHED["nc"] = build()
    return _CACHED["nc"]


def _prep_in_maps(inputs):
    gf = lambda k: np.asarray(inputs[k], np.float32)
    x = gf("x")
    wq, wk, wv = gf("wq"), gf("wk"), gf("wv")
    tobf = lambda a: np.ascontiguousarray(a).astype(ml_dtypes.bfloat16)
    col = lambda a: np.ascontiguousarray(gf(a).reshape(-1, 1))
    tri01 = (np.arange(128)[:, None] > np.arange(128)[None, :]).astype(
        ml_dtypes.bfloat16)
    tri2_np = np.ascontiguousarray(np.stack([tri01, tri01], axis=1))
    shared = dict(wo=tobf(gf("wo")), w1=tobf(gf("w1")), w2=tobf(gf("w2")),
                  bo_c=col("bo"), b1_c=col("b1"), b2_c=col("b2"),
                  g1_c=col("ln1_g"), bln1_c=col("ln1_b"),
                  g2_c=col("ln2_g"), bln2_c=col("ln2_b"), tri2=tri2_np)
    in_maps = []
    for c in range(NCORES):
        b, q = c // 4, c % 4
        m = dict(shared)
        m["xT"] = np.ascontiguousarray(x[b, q * R:(q + 1) * R, :].T)
        m["wq_my"] = tobf(wq[:, 128 * c:128 * (c + 1)])
        m["wk_my"] = tobf(wk[:, 128 * c:128 * (c + 1)])
        m["wv_my"] = tobf(wv[:, 128 * c:128 * (c + 1)])
        in_maps.append(m)
    return in_maps


def run(inputs, **kw):
    nc = _get_nc()
    in_maps = _prep_in_maps(inputs)
    res = bass_utils.run_bass_kernel_spmd(nc, in_maps,
                                          core_ids=list(range(NCORES)), **kw)
    out = np.empty((B, S, D), np.float32)
    for c in range(NCORES):
        b, q = c // 4, c % 4
        out[b, q * R:(q + 1) * R, :] = res.results[c]["outT"].T
    return out, res


def kernel(**inputs):
    out, _ = run(inputs)
    return out


# revision 11
# speedup vs baseline: 1.2633x; 1.0047x over previous
"""Trainium2 Bass kernel for a decoder layer (LN->attn->res, LN->attn->FFN->res).

Sharding (8 cores, SPMD single program):
 - Row-parallel phases (LN / out-proj / FFN / residuals): global rows B*S =
   4096 split 512/core: core c owns batch c//4, seq rows [512*(c%4), +512).
 - Attention head-parallel: core c computes 2 global heads {2c, 2c+1} for
   both batches over the full sequence (per-core column-sliced QKV weights).
 - Collectives: one 8-core AllGather of the LN output per attention (the
   cheap direction: 1 MB in / 7 MB received), and one AllToAll of the
   attention outputs (1 MB, ~8x less wire than the AllGather equivalent)
   feeding a local out-projection.
 - Dummy PE matmuls are streamed during both collective windows to keep the
   HAM clock gate at 2.4 GHz (otherwise the post-collective matmuls run at
   1.2 GHz for ~3.4 us).

Layout: activations feature-on-partition ([D, rows]); host pre-transposes x
and post-transposes the output. Matmuls bf16, accumulation fp32, residual fp32.

Masking: reference masks k <= q (strictly-upper attention). Softmax runs
without max-subtraction (scores bounded); both heads' score tiles share one
2-bank PSUM tile so a single Exp covers them; diagonal 128x128 blocks are
masked multiplicatively AFTER exp with a 0/1 upper-strict mask on the vector
engine; fully-masked tiles are skipped via per-tile column prefixes. Softmax
denominators ride in PSUM row 64 via a ones-column appended to V; they are
reshaped to [128,8] through DRAM before the reciprocal (DVE reciprocal is
~6.4ns/elem along the free dim). The all-masked last row (uniform attention
over all 2048 keys) is patched post-normalize with a DVE reduce of V.

LayerNorm stats use a [128,128] ones stationary so every per-row scalar op
runs on all 128 lanes.
"""

import numpy as np
import ml_dtypes

import concourse.bass as bass
import concourse.bacc as bacc
import concourse.tile as tile
import concourse.mybir as mybir
from concourse import bass_utils

B, S, D, H, F = 2, 2048, 1024, 16, 4096
HD = D // H            # 64
NCORES = 8
R = S // 4             # 512 rows per core
KT16 = S // 128        # 16 seq tiles
DT8 = D // 128         # 8 feature tiles of D
FT32 = F // 128        # 32 feature tiles of F
EPS = 1e-5

f32 = mybir.dt.float32
bf16 = mybir.dt.bfloat16
GROUP8 = [list(range(8))]
AF = mybir.ActivationFunctionType
ALU = mybir.AluOpType


def _emit(nc, tc, ctxpools):
    # ---------------- I/O tensors ----------------
    xT = nc.dram_tensor("xT", [D, R], f32, kind="ExternalInput")
    wq_my = nc.dram_tensor("wq_my", [D, 2 * HD], bf16, kind="ExternalInput")
    wk_my = nc.dram_tensor("wk_my", [D, 2 * HD], bf16, kind="ExternalInput")
    wv_my = nc.dram_tensor("wv_my", [D, 2 * HD], bf16, kind="ExternalInput")
    wo = nc.dram_tensor("wo", [D, D], bf16, kind="ExternalInput")
    w1 = nc.dram_tensor("w1", [D, F], bf16, kind="ExternalInput")
    w2 = nc.dram_tensor("w2", [F, D], bf16, kind="ExternalInput")
    bo_c = nc.dram_tensor("bo_c", [D, 1], f32, kind="ExternalInput")
    b1_c = nc.dram_tensor("b1_c", [F, 1], f32, kind="ExternalInput")
    b2_c = nc.dram_tensor("b2_c", [D, 1], f32, kind="ExternalInput")
    g1_c = nc.dram_tensor("g1_c", [D, 1], f32, kind="ExternalInput")
    bln1_c = nc.dram_tensor("bln1_c", [D, 1], f32, kind="ExternalInput")
    g2_c = nc.dram_tensor("g2_c", [D, 1], f32, kind="ExternalInput")
    bln2_c = nc.dram_tensor("bln2_c", [D, 1], f32, kind="ExternalInput")
    tri2 = nc.dram_tensor("tri2", [128, 2, 128], bf16, kind="ExternalInput")
    outT = nc.dram_tensor("outT", [D, R], f32, kind="ExternalOutput")

    const = ctxpools["const"]
    persist = ctxpools["persist"]
    dram = ctxpools["dram"]

    # ---------------- persistent activations ----------------
    resT = persist.tile([128, DT8, R], f32)    # residual stream (fp32)
    hT1 = persist.tile([128, DT8, R], bf16)    # LN output / attn input
    h2T = persist.tile([128, DT8, R], bf16)    # attn2 out -> FFN in
    OT = persist.tile([128, DT8, R], bf16)     # gathered attention out

    # x first: LN1 is the first consumer
    nc.sync.dma_start(out=resT, in_=xT[:, :].rearrange("(k p) q -> p k q", p=128))

    # ---------------- constants / weights resident in SBUF ----------------
    g1_sb = const.tile([128, DT8], f32)
    nc.sync.dma_start(out=g1_sb, in_=g1_c[:, 0:1].rearrange("(k p) o -> p (k o)", p=128))
    bln1_sb = const.tile([128, DT8], f32)
    nc.sync.dma_start(out=bln1_sb, in_=bln1_c[:, 0:1].rearrange("(k p) o -> p (k o)", p=128))
    g2_sb = const.tile([128, DT8], f32)
    nc.sync.dma_start(out=g2_sb, in_=g2_c[:, 0:1].rearrange("(k p) o -> p (k o)", p=128))
    bln2_sb = const.tile([128, DT8], f32)
    nc.sync.dma_start(out=bln2_sb, in_=bln2_c[:, 0:1].rearrange("(k p) o -> p (k o)", p=128))
    bo_sb = const.tile([128, DT8], f32)
    nc.sync.dma_start(out=bo_sb, in_=bo_c[:, 0:1].rearrange("(k p) o -> p (k o)", p=128))
    b1_sb = const.tile([128, FT32], f32)
    nc.sync.dma_start(out=b1_sb, in_=b1_c[:, 0:1].rearrange("(k p) o -> p (k o)", p=128))
    b2_sb = const.tile([128, DT8], f32)
    nc.sync.dma_start(out=b2_sb, in_=b2_c[:, 0:1].rearrange("(k p) o -> p (k o)", p=128))
    tri2_sb = const.tile([128, 2, 128], bf16)
    nc.sync.dma_start(out=tri2_sb, in_=tri2[:, :, :])
    ones128 = const.tile([128, 128], bf16)
    nc.vector.memset(ones128, 1.0)
    eps_sb = const.tile([128, 1], f32)
    nc.vector.memset(eps_sb, EPS)
    wrm = const.tile([128, R], bf16)
    nc.vector.memset(wrm, 0.25)
    wq_sb = const.tile([128, DT8, 2 * HD], bf16)
    nc.sync.dma_start(out=wq_sb, in_=wq_my[:, :].rearrange("(k p) n -> p k n", p=128))
    wk_sb = const.tile([128, DT8, 2 * HD], bf16)
    nc.sync.dma_start(out=wk_sb, in_=wk_my[:, :].rearrange("(k p) n -> p k n", p=128))
    wv_sb = const.tile([128, DT8, 2 * HD], bf16)
    nc.sync.dma_start(out=wv_sb, in_=wv_my[:, :].rearrange("(k p) n -> p k n", p=128))
    wo_sb = const.tile([128, DT8, D], bf16)
    nc.sync.dma_start(out=wo_sb, in_=wo[:, :].rearrange("(k p) n -> p k n", p=128))

    warm_ctr = [0]

    def warm_pe(n):
        # dead matmuls that keep the HAM clock gate open through a
        # collective window; sunk into a DRAM scratch to survive DCE
        warm_ctr[0] += 1
        with (
            tc.tile_pool(name=f"warm{warm_ctr[0]}", bufs=1) as wp,
            tc.tile_pool(name=f"warm_ps{warm_ctr[0]}", bufs=1,
                         space="PSUM") as wps,
        ):
            psw = wps.tile([128, R], f32)
            for _ in range(n):
                nc.tensor.matmul(psw, ones128, wrm, start=True, stop=True)
            sink = wp.tile([128, R], bf16)
            nc.vector.tensor_copy(out=sink, in_=psw)
            scr = dram.tile([128, R], bf16, tag="warmscr", bufs=2)
            nc.sync.dma_start(out=scr, in_=sink)

    # =================================================================
    def layernorm(g_sb, bln_sb, hT):
        with (
            tc.tile_pool(name="ln_sb", bufs=2) as lnp,
            tc.tile_pool(name="ln_ps", bufs=1, space="PSUM") as lnps,
        ):
            ps_sum = lnps.tile([128, R], f32, tag="s")
            ps_sq = lnps.tile([128, R], f32, tag="q")
            for kt in range(DT8):
                rb = lnp.tile([128, R], bf16, tag="rb", bufs=3)
                nc.vector.tensor_copy(out=rb, in_=resT[:, kt, :])
                sq = lnp.tile([128, R], bf16, tag="sq", bufs=3)
                nc.vector.tensor_mul(out=sq, in0=rb, in1=rb)
                nc.tensor.matmul(ps_sum, ones128, rb,
                                 start=(kt == 0), stop=(kt == DT8 - 1))
                nc.tensor.matmul(ps_sq, ones128, sq,
                                 start=(kt == 0), stop=(kt == DT8 - 1))
            # all per-row scalars live on all 128 partitions (full DVE width)
            mu = lnp.tile([128, R], f32)
            nc.vector.tensor_scalar_mul(out=mu, in0=ps_sum, scalar1=1.0 / D)
            msq = lnp.tile([128, R], f32)
            nc.vector.tensor_scalar_mul(out=msq, in0=ps_sq, scalar1=1.0 / D)
            mu2 = lnp.tile([128, R], f32)
            nc.vector.tensor_mul(out=mu2, in0=mu, in1=mu)
            var = lnp.tile([128, R], f32)
            nc.vector.tensor_tensor(out=var, in0=msq, in1=mu2, op=ALU.subtract)
            sd = lnp.tile([128, R], f32)
            nc.scalar.activation(out=sd, in_=var, func=AF.Sqrt, bias=eps_sb)
            rstd = lnp.tile([128, R], f32)
            nc.vector.reciprocal(out=rstd, in_=sd)
            for kt in range(DT8):
                t1 = lnp.tile([128, R], f32, tag="t1", bufs=3)
                nc.vector.tensor_tensor(out=t1, in0=resT[:, kt, :],
                                        in1=mu, op=ALU.subtract)
                t2 = lnp.tile([128, R], f32, tag="t2", bufs=3)
                nc.vector.tensor_mul(out=t2, in0=t1, in1=rstd)
                nc.vector.tensor_scalar(out=hT[:, kt, :], in0=t2,
                                        scalar1=g_sb[:, kt:kt + 1],
                                        scalar2=bln_sb[:, kt:kt + 1],
                                        op0=ALU.mult, op1=ALU.add)

    # =================================================================
    def attention(hT, first):
        # AllGather the LN output; each core then computes q/k/v for its own
        # 2 heads over the full sequence from host-sliced weight columns.
        ag_in = dram.tile([D, R], bf16, tag="ag1i", bufs=2)
        ag_out = dram.tile([NCORES * D, R], bf16, addr_space="Shared",
                           tag="ag1o", bufs=2)
        nc.sync.dma_start(
            out=ag_in.rearrange("(k p) q -> p k q", p=128), in_=hT)
        nc.gpsimd.collective_compute(
            "AllGather", ALU.bypass, replica_groups=GROUP8,
            ins=[ag_in.opt()], outs=[ag_out.opt()])
        ag1v = ag_out.rearrange("(r k p) q -> r k p q", r=NCORES, k=DT8, p=128)
        warm_pe(110)

        a2a_o_in = dram.tile([NCORES * 128, R], bf16, tag="a2aoi", bufs=2)
        a2a_o_out = dram.tile([NCORES * 128, R], bf16, tag="a2aoo", bufs=2)
        with (
            tc.tile_pool(name="at_sb", bufs=2) as ap,
            tc.tile_pool(name="at_big", bufs=2) as bigp,
        ):
            for b in range(B):
                # ---- gather LN output for batch b: [128, kt, 2048] ----
                hfull = bigp.tile([128, DT8, S], bf16, tag="hfull")
                for kt in range(DT8):
                    nc.sync.dma_start(
                        out=hfull[:, kt, :].rearrange("p (r q) -> p r q", r=4),
                        in_=ag1v[4 * b:4 * b + 4, kt].rearrange(
                            "r p q -> p r q"))

                # ---- QKV for my 2 heads, full sequence ----
                qT = ap.tile([128, 4, R], bf16, tag="qT")
                kT = ap.tile([128, 4, R], bf16, tag="kT")
                vT = ap.tile([128, 4, R], bf16, tag="vT")
                with tc.tile_pool(name="qkv_ps", bufs=4, space="PSUM") as qps:
                    for dst, wsb, scale in ((kT, wk_sb, None),
                                            (qT, wq_sb, 0.125),
                                            (vT, wv_sb, None)):
                        for c in range(4):
                            ps = qps.tile([128, R], f32, tag="ps")
                            for kt in range(DT8):
                                nc.tensor.matmul(
                                    ps, wsb[:, kt, :],
                                    hfull[:, kt, c * R:(c + 1) * R],
                                    start=(kt == 0), stop=(kt == DT8 - 1))
                            if scale is None:
                                nc.vector.tensor_copy(out=dst[:, c, :], in_=ps)
                            else:
                                nc.vector.tensor_scalar_mul(
                                    out=dst[:, c, :], in0=ps, scalar1=scale)

                # ---- V transpose -> v_aug [128(seq), kt, h, 65] ----
                vaug = ap.tile([128, KT16, 2, HD + 1], bf16, tag="vaug")
                nc.vector.memset(vaug[:, :, :, HD:HD + 1], 1.0)
                for kt in range(KT16):
                    vtmp = ap.tile([128, 128], bf16, tag="vtmp", bufs=4)
                    nc.sync.dma_start(
                        out=vtmp,
                        in_=vT[:, kt // 4, (kt % 4) * 128:(kt % 4) * 128 + 128],
                        transpose=True)
                    nc.vector.tensor_copy(out=vaug[:, kt, 0, 0:HD],
                                          in_=vtmp[:, 0:HD])
                    nc.vector.tensor_copy(out=vaug[:, kt, 1, 0:HD],
                                          in_=vtmp[:, HD:128])
                # mean of V over the whole sequence (all-masked last row)
                sumv = ap.tile([128, 1], f32, tag="sumv")
                nc.vector.tensor_reduce(out=sumv, in_=vT,
                                        axis=mybir.AxisListType.XY, op=ALU.add)

                oT = ap.tile([128, 4, R], bf16, tag="oT")
                with (
                    tc.tile_pool(name="sc_ps", bufs=2, space="PSUM") as scps,
                    tc.tile_pool(name="av_ps", bufs=4, space="PSUM") as avps,
                ):
                    for c in range(4):
                        po = [avps.tile([128, R], f32, tag="po", name=f"po{hh}")
                              for hh in range(2)]
                        for kt in range(KT16 - 1, 4 * c - 1, -1):
                            npfx = min(kt - 4 * c + 1, 4) * 128
                            pss = scps.tile([128, 2, R], f32, tag="pss")
                            for hh in range(2):
                                lo = hh * HD
                                nc.tensor.matmul(
                                    pss[:, hh, 0:npfx],
                                    kT[lo:lo + HD, kt // 4,
                                       (kt % 4) * 128:(kt % 4) * 128 + 128],
                                    qT[lo:lo + HD, c, 0:npfx],
                                    start=True, stop=True)
                            wt = ap.tile([128, 2, R], bf16, tag="wt", bufs=3)
                            nc.scalar.activation(out=wt[:, :, 0:npfx],
                                                 in_=pss[:, :, 0:npfx],
                                                 func=AF.Exp)
                            if kt <= 4 * c + 3:
                                # diagonal block: exact multiplicative mask
                                nc.vector.tensor_mul(
                                    out=wt[:, :, npfx - 128:npfx],
                                    in0=wt[:, :, npfx - 128:npfx],
                                    in1=tri2_sb)
                            for hh in range(2):
                                nc.tensor.matmul(
                                    po[hh][0:HD + 1, 0:npfx],
                                    vaug[:, kt, hh, :],
                                    wt[:, hh, 0:npfx],
                                    start=(kt == KT16 - 1),
                                    stop=(kt == 4 * c))
                        # Drain po to SBUF immediately (~1us) so the PSUM
                        # banks recycle; the normalize then runs off the SBUF
                        # copy out-of-band. Denominators (row HD) are
                        # reshaped to [128, 8] through DRAM before inverting
                        # (DVE reciprocal is ~6.4ns/elem along the free dim),
                        # then broadcast.
                        poc = ap.tile([HD + 1, 2, R], f32, tag="poc", bufs=3)
                        for hh in range(2):
                            nc.vector.tensor_copy(out=poc[:, hh, :],
                                                  in_=po[hh][0:HD + 1, :])
                        scr2 = dram.tile([1, 2, R], f32, tag="rscr", bufs=4)
                        nc.sync.dma_start(out=scr2, in_=poc[HD:HD + 1, :, :])
                        denp = ap.tile([128, 8], f32, tag="denp", bufs=4)
                        nc.sync.dma_start(
                            out=denp,
                            in_=scr2[0].rearrange("h (p f) -> (h p) f", p=64))
                        recp = ap.tile([128, 8], f32, tag="recp", bufs=4)
                        nc.vector.reciprocal(out=recp, in_=denp)
                        scr3 = dram.tile([1, 2, R], f32, tag="rscr3", bufs=4)
                        nc.sync.dma_start(
                            out=scr3[0].rearrange("h (p f) -> (h p) f", p=64),
                            in_=recp)
                        recb = ap.tile([HD, 2, R], f32, tag="recb", bufs=2)
                        nc.sync.dma_start(
                            out=recb,
                            in_=scr3[0][None].broadcast_to([HD, 2, R]))
                        for hh in range(2):
                            nc.vector.tensor_mul(
                                out=oT[hh * HD:hh * HD + HD, c, :],
                                in0=poc[0:HD, hh, :], in1=recb[:, hh, :])
                        if c == 3:
                            # all-masked last row: uniform attention = mean(V)
                            nc.vector.tensor_scalar_mul(
                                out=oT[:, 3, R - 1:R], in0=sumv,
                                scalar1=1.0 / S)
                nc.sync.dma_start(
                    out=a2a_o_in[bass.ds(4 * b * 128, 512), :].rearrange(
                        "(c p) q -> p c q", c=4),
                    in_=oT)
        nc.gpsimd.collective_compute(
            "AllToAll", ALU.bypass, replica_groups=GROUP8,
            ins=[a2a_o_in.opt()], outs=[a2a_o_out.opt()])
        warm_pe(70)
        nc.sync.dma_start(
            out=OT, in_=a2a_o_out[:, :].rearrange("(s p) q -> p s q", p=128))

        # ---- out-projection (+ bias, + residual or -> h2T) ----
        with (
            tc.tile_pool(name="op_sb", bufs=3) as opp,
            tc.tile_pool(name="op_ps", bufs=4, space="PSUM") as opps,
        ):
            for nt in range(DT8):
                ps = opps.tile([128, R], f32, tag="ps")
                for s in range(DT8):
                    nc.tensor.matmul(ps, wo_sb[:, s, nt * 128:nt * 128 + 128],
                                     OT[:, s, :],
                                     start=(s == 0), stop=(s == DT8 - 1))
                if first:
                    t = opp.tile([128, R], f32, tag="t")
                    nc.vector.tensor_scalar(out=t, in0=ps,
                                            scalar1=bo_sb[:, nt:nt + 1],
                                            scalar2=None, op0=ALU.add)
                    nc.vector.tensor_add(out=resT[:, nt, :],
                                         in0=resT[:, nt, :], in1=t)
                else:
                    nc.vector.tensor_scalar(out=h2T[:, nt, :], in0=ps,
                                            scalar1=bo_sb[:, nt:nt + 1],
                                            scalar2=None, op0=ALU.add)

    # =================================================================
    # layer body
    layernorm(g1_sb, bln1_sb, hT1)
    attention(hT1, first=True)
    layernorm(g2_sb, bln2_sb, hT1)
    attention(hT1, first=False)

    # ---- FFN ----
    with (
        tc.tile_pool(name="ffn_sb", bufs=3) as fp,
        tc.tile_pool(name="ffn_big", bufs=1) as fbig,
        tc.tile_pool(name="ffn_ps", bufs=4, space="PSUM") as fps,
    ):
        gT = fbig.tile([128, FT32, R], bf16)
        w1v = w1[:, :].rearrange("(k p) (nt n) -> nt k p n", p=128, n=128)
        for nt in range(FT32):
            w1t = fp.tile([128, DT8, 128], bf16, tag="w1t", bufs=4)
            nc.sync.dma_start(out=w1t, in_=w1v[nt].rearrange("k p n -> p k n"))
            ps = fps.tile([128, R], f32, tag="ps1")
            for kt in range(DT8):
                nc.tensor.matmul(ps, w1t[:, kt, :], h2T[:, kt, :],
                                 start=(kt == 0), stop=(kt == DT8 - 1))
            nc.scalar.activation(out=gT[:, nt, :], in_=ps, func=AF.Gelu,
                                 bias=b1_sb[:, nt:nt + 1])
        w2v = w2[:, :].rearrange("(k p) (nt n) -> nt k p n", p=128, n=128)
        for nt in range(DT8):
            w2t = fp.tile([128, FT32, 128], bf16, tag="w2t", bufs=3)
            nc.sync.dma_start(out=w2t, in_=w2v[nt].rearrange("k p n -> p k n"))
            ps2 = fps.tile([128, R], f32, tag="ps2")
            for kt in range(FT32):
                nc.tensor.matmul(ps2, w2t[:, kt, :], gT[:, kt, :],
                                 start=(kt == 0), stop=(kt == FT32 - 1))
            t = fp.tile([128, R], f32, tag="t")
            nc.vector.tensor_scalar(out=t, in0=ps2,
                                    scalar1=b2_sb[:, nt:nt + 1],
                                    scalar2=None, op0=ALU.add)
            ot = fp.tile([128, R], f32, tag="ot")
            nc.vector.tensor_add(out=ot, in0=t, in1=resT[:, nt, :])
            nc.sync.dma_start(out=outT[nt * 128:(nt + 1) * 128, :], in_=ot)


def build():
    nc = bacc.Bacc("TRN2", target_bir_lowering=False, debug=False,
                   num_devices=NCORES)
    with tile.TileContext(nc) as tc:
        with (
            tc.tile_pool(name="const", bufs=1) as const,
            tc.tile_pool(name="persist", bufs=1) as persist,
            tc.tile_pool(name="dram", bufs=1, space="DRAM") as dram,
        ):
            _emit(nc, tc, {"const": const, "persist": persist, "dram": dram})
    nc.compile()
    return nc


_CACHED = {}


def _get_nc():
    if "nc" not in _CACHED:
        _CACHED["nc"] = build()
    return _CACHED["nc"]


def _prep_in_maps(inputs):
    gf = lambda k: np.asarray(inputs[k], np.float32)
    x = gf("x")
    wq, wk, wv = gf("wq"), gf("wk"), gf("wv")
    tobf = lambda a: np.ascontiguousarray(a).astype(ml_dtypes.bfloat16)
    col = lambda a: np.ascontiguousarray(gf(a).reshape(-1, 1))
    tri01 = (np.arange(128)[:, None] > np.arange(128)[None, :]).astype(
        ml_dtypes.bfloat16)
    tri2_np = np.ascontiguousarray(np.stack([tri01, tri01], axis=1))
    shared = dict(wo=tobf(gf("wo")), w1=tobf(gf("w1")), w2=tobf(gf("w2")),
                  bo_c=col("bo"), b1_c=col("b1"), b2_c=col("b2"),
                  g1_c=col("ln1_g"), bln1_c=col("ln1_b"),
                  g2_c=col("ln2_g"), bln2_c=col("ln2_b"), tri2=tri2_np)
    in_maps = []
    for c in range(NCORES):
        b, q = c // 4, c % 4
        m = dict(shared)
        m["xT"] = np.ascontiguousarray(x[b, q * R:(q + 1) * R, :].T)
        m["wq_my"] = tobf(wq[:, 128 * c:128 * (c + 1)])
        m["wk_my"] = tobf(wk[:, 128 * c:128 * (c + 1)])
        m["wv_my"] = tobf(wv[:, 128 * c:128 * (c + 1)])
        in_maps.append(m)
    return in_maps


def run(inputs, **kw):
    nc = _get_nc()
    in_maps = _prep_in_maps(inputs)
    res = bass_utils.run_bass_kernel_spmd(nc, in_maps,
                                          core_ids=list(range(NCORES)), **kw)
    out = np.empty((B, S, D), np.float32)
    for c in range(NCORES):
        b, q = c // 4, c % 4
        out[b, q * R:(q + 1) * R, :] = res.results[c]["outT"].T
    return out, res


def kernel(**inputs):
    out, _ = run(inputs)
    return out
